# revision 15
# baseline (speedup 1.0000x reference)
"""GQA attention (B=2, S=2048, HID=2048, 32 q heads / 8 kv heads, fp32 I/O)
on 8 TRN2 NeuronCores.

Sharding: sequence-parallel with fully local K/V. Core c owns 512 query
tokens of batch c//4 (cores 0-3 = batch 0, cores 4-7 = batch 1), but
computes K^T and V for ALL 2048 tokens of its batch locally — that
(+~40% KV projection FLOPs) is much cheaper than an intra-chip
AllGather, which measures 100-170us and blockades the DMA engines while
it runs. Attention is permutation-invariant over keys, so each core
orders tokens own-block-first (host-side permutation) and the device
program stays rank-independent.

V carries a fused ones-column per kv head so the PV matmul also
produces the softmax row-sums; the output-projection bias is fused as
an extra contraction row. All matmuls run in bf16 with fp32 PSUM
accumulation. Score matmuls (K=64) pair head A (partitions 0-63) and
head B (64-127) on disjoint PE row-groups so the hardware co-executes
them (auto tile_position from base partitions).

v2 schedule: the serial K/V-projection prologue is collapsed to one
8-bank PSUM wave {K mt0 x4, Q0, Q2, V(c0,kv0-3), V(c1,kv0-3)} streamed
chunk-outer against the x DMA arrival, so the first exp fires at
~45us (was ~122us). Everything else — V in 30 finer (chunk, kv-half)
units, K mt1-mt3, Q chunks, Wo prefetch — drips into the ACT-bound
pair loop through a 2-bank ping-pong PSUM ring with deadline-forced,
cost-budgeted pops. The xin pool (x remainder + Wk/Wv) releases at
step 96 to make room for prefetching all 4 Wo column blocks, so the
output projection runs as a pure-PE tail with all 8 PSUM banks.
"""

import functools
from collections import deque
from contextlib import ExitStack

import numpy as np
import ml_dtypes

import concourse.bass as bass
import concourse.mybir as mybir
import concourse.tile as tile
from concourse import bacc
from concourse.bass_utils import run_bass_kernel_spmd

BF = mybir.dt.bfloat16
F32 = mybir.dt.float32

B, S, HID = 2, 2048, 2048
NH, NKV, HD = 32, 8, 64          # q heads, kv heads, head dim
GRP = NH // NKV                  # 4 q heads per kv head
TP = 4                           # cores per batch group
TOK = S // TP                    # 512 local query tokens per core
KC = HID // 128                  # 16 contraction chunks of 128
NKC = S // 128                   # 16 key chunks of 128 (full seq)
VW = NKV * (HD + 1)              # 520: V width incl. ones columns
EXP_SCALE = float(HD) ** -0.5    # 1/8 softmax scale, fused into Exp
LAG = 2                          # steps between scores+exp and its PV


def q_slot(h):
    """qTp tile index and partition base for head h.

    Head h lives at partition base ((h//4)%2)*64 — the same base its kv
    head kh=h//4 occupies inside the kTg tiles, so the scores matmul's
    lhsT and rhs stay partition-aligned (and heads A/B co-execute on
    disjoint PE row groups).
    """
    return ((h // 4) // 2) * 4 + (h % 4), ((h // 4) % 2) * 64


def build_graph():
    nc = bacc.Bacc(None, target_bir_lowering=False, debug=False, num_devices=8)

    xT = nc.declare_dram_parameter("xT", [HID, S], BF, isOutput=False)
    wkT = nc.declare_dram_parameter("wkT", [HID, NKV * HD], BF, isOutput=False)
    wvT = nc.declare_dram_parameter("wvT", [HID, NKV * HD], BF, isOutput=False)
    wqT = nc.declare_dram_parameter("wqT", [HID, HID], BF, isOutput=False)
    woT = nc.declare_dram_parameter("woT", [HID + 1, HID], BF, isOutput=False)
    out = nc.declare_dram_parameter("out", [TOK, HID], F32, isOutput=True)

    with tile.TileContext(nc) as tc, ExitStack() as es:
        pers = es.enter_context(tc.tile_pool(name="pers", bufs=1))

        def T(shape, dtype, *, name):
            return pers.tile(shape, dtype, name=name, tag=name)

        # long-lived SBUF pools first; xin LAST so it can release at
        # step 96 while still top-of-stack among SBUF pools.
        wqp = es.enter_context(tc.tile_pool(name="wqp", bufs=48))
        stgB = es.enter_context(tc.tile_pool(name="stgB", bufs=2))
        nrm = es.enter_context(tc.tile_pool(name="nrm", bufs=2))
        ptp = es.enter_context(tc.tile_pool(name="ptp", bufs=6))

        xq = [T([128, TOK], BF, name=f"xq{k}") for k in range(KC)]

        xin_cm = tc.tile_pool(name="xin", bufs=1)
        xin = xin_cm.__enter__()
        xr = [xin.tile([128, S - TOK], BF, tag=f"xr{k}", name=f"xr{k}")
              for k in range(KC)]
        wk_sb = [xin.tile([128, NKV * HD], BF, tag=f"wk{k}", name=f"wk{k}")
                 for k in range(KC)]
        wv_sb = [xin.tile([128, NKV * HD], BF, tag=f"wv{k}", name=f"wv{k}")
                 for k in range(KC)]

        # weight tiles for wave-0's Q0/Q2 — allocated before the DMA loop
        wq_w0 = {m: [wqp.tile([128, 128], BF, tag="wq", name=f"wq{m}_{k}")
                     for k in range(KC)] for m in (0, 2)}

        # ---- DMA issue order = priority: everything wave 0 needs, by
        # contraction chunk, then wv (first needed ~5us after wave 0).
        for k in range(KC):
            nc.sync.dma_start(out=wk_sb[k][:, :], in_=wkT[k * 128:(k + 1) * 128, :])
            nc.sync.dma_start(out=xq[k][:, :], in_=xT[k * 128:(k + 1) * 128, 0:TOK])
            nc.sync.dma_start(out=xr[k][:, :], in_=xT[k * 128:(k + 1) * 128, TOK:S])
            # wv INSIDE the chunk loop: wave 0's V matmuls consume
            # wv_sb[k] at chunk-k arrival (the in-order PE queue would
            # otherwise stall the whole wave on a late wv).
            nc.sync.dma_start(out=wv_sb[k][:, :], in_=wvT[k * 128:(k + 1) * 128, :])
            for m in (0, 2):
                nc.sync.dma_start(
                    out=wq_w0[m][k][:, :],
                    in_=wqT[k * 128:(k + 1) * 128, m * 128:(m + 1) * 128])

        def xcols(k, lo, n):
            # columns lo..lo+n of the permuted x^T chunk k
            return xq[k][:, lo:lo + n] if lo < TOK \
                else xr[k][:, lo - TOK:lo - TOK + n]

        # row HD (partition 64) is the K=1 lhsT for the row-sum broadcast
        ones64 = T([HD + 1, 64], BF, name="ones64")
        nc.vector.memset(ones64[:, :], 1.0)
        ones128 = T([1, 128], BF, name="ones128")
        nc.vector.memset(ones128[:, :], 1.0)

        # kTg[nb*4+mt]: [128, 512] = K^T rows mt*128.. for key block nb
        # (kv heads 2mt at partitions 0-63, 2mt+1 at 64-127).
        # vg[c]: [128, 520] V_aug rows for key chunk c, ones at col
        # kh*65+64 of each kv head kh.
        kTg = [T([128, TOK], BF, name=f"kTg{i}") for i in range(16)]
        vg = [T([128, VW], BF, name=f"vg{c}") for c in range(NKC)]
        qTp = [T([128, TOK], BF, name=f"qTp{i}") for i in range(NH // 2)]
        attnT = [T([128, TOK], BF, name=f"attnT{t}") for t in range(NH // 2)]

        def q_fin(ps, m):
            st = stgB.tile([128, TOK], BF, tag="stg", name=f"stq{m}")
            nc.vector.tensor_copy(out=st[:, :], in_=ps[:, :])
            for j in range(2):
                h = 2 * m + j
                i, roff = q_slot(h)
                nc.sync.dma_start(out=qTp[i][roff:roff + 64, :],
                                  in_=st[j * 64:(j + 1) * 64, :])

        def v_fin(ps, c, h):
            if h == 0:
                nc.vector.memset(vg[c][:, :], 1.0)
            for kh in range(4 * h, 4 * h + 4):
                nc.vector.tensor_copy(
                    out=vg[c][:, kh * (HD + 1):kh * (HD + 1) + HD],
                    in_=ps[:, (kh - 4 * h) * 64:(kh - 4 * h + 1) * 64])

        # =============== wave 0: the minimal exp-gating work ============
        # 8 PSUM accumulation groups, contraction-chunk OUTER so the PE
        # streams 8 matmuls per arriving x chunk. Completes ~1.7us after
        # the last x chunk lands; first exp fires ~2us later.
        w0 = ([("k", 0, nb) for nb in range(TP)]
              + [("q", 0, None), ("q", 2, None)]
              + [("v", 0, 0), ("v", 1, 0)])
        with tc.tile_pool(name="accA", bufs=8, space="PSUM") as accA:
            pss = [accA.tile([128, TOK], F32, tag="acc", name=f"psA{i}")
                   for i in range(8)]
            for k in range(KC):
                for ps, (kind, a, b) in zip(pss, w0):
                    if kind == "k":
                        nc.tensor.matmul(
                            out=ps[:, :],
                            lhsT=wk_sb[k][:, 0:128],
                            rhs=xcols(k, b * TOK, TOK),
                            start=(k == 0), stop=(k == KC - 1))
                    elif kind == "q":
                        nc.tensor.matmul(
                            out=ps[:, :], lhsT=wq_w0[a][k][:, :],
                            rhs=xq[k][:, :],
                            start=(k == 0), stop=(k == KC - 1))
                    else:
                        nc.tensor.matmul(
                            out=ps[:, 0:256],
                            lhsT=xcols(k, a * 128, 128),
                            rhs=wv_sb[k][:, 0:256],
                            start=(k == 0), stop=(k == KC - 1))
            # evac in exp-gating order: kTg nb0, Q0, Q2, then the rest
            nc.vector.tensor_copy(out=kTg[0][:, :], in_=pss[0][:, :])
            q_fin(pss[4], 0)
            q_fin(pss[5], 2)
            for nb in range(1, TP):
                nc.vector.tensor_copy(out=kTg[nb * 4][:, :],
                                      in_=pss[nb][:, :])
            v_fin(pss[6][:, 0:256], 0, 0)
            v_fin(pss[7][:, 0:256], 1, 0)

        # =============== drip units (fed into the pair loop) ============
        # Each unit: (deadline_step, [(cost_ns, thunk), ...]).
        MM_NS = 216    # 512-col bf16 matmul streaming time
        VMM_NS = 112   # 256-col

        drip = deque()  # (deadline, cost_ns, thunk)
        _units = []     # (deadline, seq, [(cost, thunk), ...])

        def push_unit(deadline, cts):
            _units.append((deadline, len(_units), cts))

        def seal_units():
            # stable-sort by deadline so FIFO head-forcing pops in need
            # order (units are queued grouped by kind, not by deadline)
            total = 0
            for d, _, cts in sorted(_units, key=lambda u: (u[0], u[1])):
                for c, t in cts:
                    drip.append((d, c, t))
                    total += c
            return total

        # unit PSUM tiles MUST allocate lazily at pop time: the dps ring
        # orders its WAR handoffs by .tile() call order, which has to
        # match emission order (norm_bcast's pb tiles share the ring).
        def gen_q_chunk(m):
            """16 weight DMAs (6-ahead interleave), 16 matmuls, finisher."""
            box = {}
            ws = [wqp.tile([128, 128], BF, tag="wq", name=f"wq{m}_{k}")
                  for k in range(KC)]
            dmas, mms = [], []
            for k in range(KC):
                def dm(k=k, m=m):
                    nc.sync.dma_start(
                        out=ws[k][:, :],
                        in_=wqT[k * 128:(k + 1) * 128, m * 128:(m + 1) * 128])
                dmas.append((0, dm))
                def mm(k=k, m=m):
                    if k == 0:
                        box["ps"] = dps.tile([128, TOK], F32, tag="dp",
                                             name=f"psq{m}")
                    nc.tensor.matmul(
                        out=box["ps"][:, :], lhsT=ws[k][:, :], rhs=xq[k][:, :],
                        start=(k == 0), stop=(k == KC - 1))
                mms.append((MM_NS, mm))
            thunks = dmas[:6]
            for k in range(KC):
                if k + 6 < KC:
                    thunks.append(dmas[k + 6])
                thunks.append(mms[k])
            thunks.append((0, lambda m=m: q_fin(box["ps"], m)))
            return thunks

        def gen_k_unit(mt, nb):
            box = {}
            thunks = []
            for k in range(KC):
                def mm(k=k, mt=mt, nb=nb):
                    if k == 0:
                        box["ps"] = dps.tile([128, TOK], F32, tag="dp",
                                             name=f"psk{nb}_{mt}")
                    nc.tensor.matmul(
                        out=box["ps"][:, :],
                        lhsT=wk_sb[k][:, mt * 128:(mt + 1) * 128],
                        rhs=xcols(k, nb * TOK, TOK),
                        start=(k == 0), stop=(k == KC - 1))
                thunks.append((MM_NS, mm))
            def fin(mt=mt, nb=nb):
                nc.vector.tensor_copy(out=kTg[nb * 4 + mt][:, :],
                                      in_=box["ps"][:, :])
            thunks.append((0, fin))
            return thunks

        def gen_v_unit(c, h):
            box = {}
            thunks = []
            for k in range(KC):
                def mm(k=k, c=c, h=h):
                    if k == 0:
                        box["ps"] = dps.tile([128, TOK], F32, tag="dp",
                                             name=f"psv{c}_{h}")
                    nc.tensor.matmul(
                        out=box["ps"][:, 0:256],
                        lhsT=xcols(k, c * 128, 128),
                        rhs=wv_sb[k][:, h * 256:(h + 1) * 256],
                        start=(k == 0), stop=(k == KC - 1))
                thunks.append((VMM_NS, mm))
            thunks.append(
                (0, lambda c=c, h=h: v_fin(box["ps"][:, 0:256], c, h)))
            return thunks

        # Wo prefetch (DMA-only): queued when xin releases at step 96.
        wo_pre = {}
        wo2_box = []

        def gen_wo_prefetch(nt):
            thunks = []
            def last(nt=nt):
                wl = wo2_box[0].tile([1, 512], BF, tag="wolast",
                                     bufs=4, name=f"wl{nt}")
                nc.sync.dma_start(
                    out=wl[:, :], in_=woT[HID:HID + 1, nt * 512:(nt + 1) * 512])
                wo_pre[(nt, "last")] = wl
            thunks.append((0, last))
            for kc in range(KC):
                def f(kc=kc, nt=nt):
                    w = wo2_box[0].tile([128, 512], BF, tag="wo", bufs=64,
                                        name=f"wo{nt}_{kc}")
                    nc.sync.dma_start(
                        out=w[:, :],
                        in_=woT[kc * 128:(kc + 1) * 128,
                                nt * 512:(nt + 1) * 512])
                    wo_pre[(nt, kc)] = w
                thunks.append((0, f))
            return thunks

        # =============== the pair loop ==================================
        pairs = []
        for g in range(0, NKV, 2):
            for j in range(GRP):
                pairs.append((g * GRP + j, (g + 1) * GRP + j))

        bc_box = [None]  # (pool, tag) for the row-sum broadcast PSUM
        ops_cm = tc.tile_pool(name="ops", bufs=2, space="PSUM")
        ops = ops_cm.__enter__()
        with tc.tile_pool(name="sps", bufs=2, space="PSUM") as sps, \
             tc.tile_pool(name="dps", bufs=2, space="PSUM") as dps:
            bc_box[0] = (dps, "dp")

            # drip queue in deadline order
            for c in range(2, NKC):                      # V kv0-3 rest
                push_unit(max(0, c // 2 - 1), gen_v_unit(c, 0))
            push_unit(13, gen_q_chunk(1))
            push_unit(13, gen_q_chunk(3))
            for nb in range(TP):                         # K mt1
                push_unit(28 + 2 * nb, gen_k_unit(1, nb))
            push_unit(29, gen_q_chunk(4))
            push_unit(29, gen_q_chunk(6))
            push_unit(44, gen_q_chunk(5))
            push_unit(44, gen_q_chunk(7))
            for c in range(NKC):                         # V kv4-7
                push_unit(61 + c // 2, gen_v_unit(c, 1))
            for nb in range(TP):                         # K mt2
                push_unit(61 + 2 * nb, gen_k_unit(2, nb))
            push_unit(61, gen_q_chunk(8))
            push_unit(61, gen_q_chunk(10))
            push_unit(76, gen_q_chunk(9))
            push_unit(76, gen_q_chunk(11))
            for nb in range(TP):                         # K mt3
                push_unit(92 + 2 * nb, gen_k_unit(3, nb))
            push_unit(93, gen_q_chunk(12))
            push_unit(93, gen_q_chunk(14))
            push_unit(108, gen_q_chunk(13))
            push_unit(108, gen_q_chunk(15))
            # rate controller: keep cumulative pops tracking an even
            # per-step pace so the queue never falls behind (deadline
            # bursts starve ACT) nor runs dry early (an idle PE drops
            # to the mid p-state and the whole step chain slows ~60%).
            pace = seal_units() / 124.0
            spent_total = 0

            # Normalization for pair p staged across pair p+1's steps
            # (PSUM->SBUF copy, reciprocal, cast, PE ones-broadcast via
            # the dps ring, multiply) so the 3.3us DVE reciprocal never
            # blocks a PE-side consumer.
            aph_of = {}

            def norm_copy(p, ci):
                j = 0 if ci == 2 else 1
                h = pairs[p][j]
                po = po_of[p][j]
                aph = nrm.tile([HD + 1, TOK], F32, tag="aph", bufs=2,
                               name=f"aph{h}")
                nc.vector.tensor_copy(out=aph[:, :], in_=po[:, :])
                aph_of[(p, j)] = [aph, None, None]
                if ci == 4:
                    del po_of[p]

            def norm_recip(p, j):
                h = pairs[p][j]
                ent = aph_of[(p, j)]
                rcp = nrm.tile([HD + 1, TOK], F32, tag="rcp", name=f"rc{h}")
                nc.vector.reciprocal(out=rcp[HD:HD + 1, :],
                                     in_=ent[0][HD:HD + 1, :])
                ent[1] = rcp

            def norm_cast(p, j):
                h = pairs[p][j]
                ent = aph_of[(p, j)]
                rcpb = nrm.tile([HD + 1, TOK], BF, tag="rcpb", name=f"rb{h}")
                nc.vector.tensor_copy(out=rcpb[HD:HD + 1, :],
                                      in_=ent[1][HD:HD + 1, :])
                ent[1] = rcpb

            def norm_bcast(p, j):
                h = pairs[p][j]
                ent = aph_of[(p, j)]
                pool, tag = bc_box[0]
                pb = pool.tile([128, TOK], F32, tag=tag, name=f"pb{h}")
                nc.tensor.matmul(out=pb[0:64, :], lhsT=ones64[HD:HD + 1, :],
                                 rhs=ent[1][HD:HD + 1, :],
                                 start=True, stop=True)
                rb = nrm.tile([64, TOK], BF, tag="rbb", name=f"rbb{h}")
                nc.vector.tensor_copy(out=rb[:, :], in_=pb[0:64, :])
                ent[2] = rb

            def norm_mul(p, j):
                h = pairs[p][j]
                ent = aph_of[(p, j)]
                t, half = h // 2, (h % 2) * 64
                if half == 0:
                    nc.vector.tensor_mul(out=attnT[t][0:64, :],
                                         in0=ent[0][0:HD, :],
                                         in1=ent[2][:, :])
                else:
                    ah = nrm.tile([64, TOK], BF, tag="ah", name=f"ah{h}")
                    nc.vector.tensor_mul(out=ah[:, :], in0=ent[0][0:HD, :],
                                         in1=ent[2][:, :])
                    nc.sync.dma_start(out=attnT[t][64:128, :], in_=ah[:, :])
                del aph_of[(p, j)]

            def norm_stage2(p, ci):
                if ci == 4:
                    norm_recip(p, 0)
                elif ci == 6:
                    norm_recip(p, 1)
                elif ci == 8:
                    norm_cast(p, 0)
                    norm_cast(p, 1)
                elif ci == 10:
                    norm_bcast(p, 0)
                elif ci == 12:
                    norm_bcast(p, 1)
                    norm_mul(p, 0)
                elif ci == 14:
                    norm_mul(p, 1)

            def emit_scores(pi, ci):
                hA, hB = pairs[pi]
                kt = (hA // GRP) // 2
                qiA, _ = q_slot(hA)
                qiB, _ = q_slot(hB)
                psA = sps.tile([128, 2 * TOK], F32, tag="ps",
                               name=f"psA{hA}_{ci}")
                psB = sps.tile([128, 2 * TOK], F32, tag="ps",
                               name=f"psB{hB}_{ci}")
                for dc in range(2):
                    c = ci + dc
                    nb, lc = c // 4, c % 4
                    kts = kTg[nb * 4 + kt]
                    nc.tensor.matmul(
                        out=psA[:, dc * TOK:(dc + 1) * TOK],
                        lhsT=kts[0:64, lc * 128:(lc + 1) * 128],
                        rhs=qTp[qiA][0:64, :], start=True, stop=True)
                ptA = ptp.tile([128, 2 * TOK], BF, tag="pt",
                               name=f"ptA{hA}_{ci}")
                nc.scalar.activation(
                    out=ptA[:, :], in_=psA[:, :],
                    func=mybir.ActivationFunctionType.Exp, scale=EXP_SCALE)
                for dc in range(2):
                    c = ci + dc
                    nb, lc = c // 4, c % 4
                    kts = kTg[nb * 4 + kt]
                    nc.tensor.matmul(
                        out=psB[:, dc * TOK:(dc + 1) * TOK],
                        lhsT=kts[64:128, lc * 128:(lc + 1) * 128],
                        rhs=qTp[qiB][64:128, :], start=True, stop=True)
                ptB = ptp.tile([128, 2 * TOK], BF, tag="pt",
                               name=f"ptB{hB}_{ci}")
                nc.scalar.activation(
                    out=ptB[:, :], in_=psB[:, :],
                    func=mybir.ActivationFunctionType.Exp, scale=EXP_SCALE)
                return ptA, ptB

            def emit_pv(pi, ci, ptA, ptB):
                hA, hB = pairs[pi]
                khA, khB = hA // GRP, hB // GRP
                poA, poB = po_of[pi]
                for dc in range(2):
                    c = ci + dc
                    nc.tensor.matmul(
                        out=poA[:, :],
                        lhsT=vg[c][:, khA * (HD + 1):(khA + 1) * (HD + 1)],
                        rhs=ptA[:, dc * TOK:(dc + 1) * TOK],
                        start=(c == 0), stop=(c == NKC - 1))
                    nc.tensor.matmul(
                        out=poB[:, :],
                        lhsT=vg[c][:, khB * (HD + 1):(khB + 1) * (HD + 1)],
                        rhs=ptB[:, dc * TOK:(dc + 1) * TOK],
                        start=(c == 0), stop=(c == NKC - 1))

            sched = [(pi, 2 * c2) for pi in range(len(pairs))
                     for c2 in range(NKC // 2)]
            po_of = {}
            inflight = deque()

            for s, (pi, ci) in enumerate(sched):
                hA, hB = pairs[pi]
                if ci == 0:
                    poA = ops.tile([HD + 1, TOK], F32, tag="po",
                                   name=f"poA{hA}")
                    poB = ops.tile([HD + 1, TOK], F32, tag="po",
                                   name=f"poB{hB}")
                    po_of[pi] = (poA, poB)
                # PV of step s-LAG first: it never waits, so it fills the
                # window where scores-A(s) stalls on exp(s-1) freeing the
                # score-PSUM ring (the PE queue is in-order).
                if len(inflight) >= LAG:
                    emit_pv(*inflight.popleft())
                ptA, ptB = emit_scores(pi, ci)
                inflight.append((pi, ci, ptA, ptB))
                if pi > 0:
                    if ci in (2, 4):
                        norm_copy(pi - 1, ci)
                    if ci >= 4:
                        norm_stage2(pi - 1, ci)
                # deadline-forced + rate-paced drip pops
                while drip and (drip[0][0] <= s + 2
                                or spent_total < (s + 1) * pace):
                    _, cost, th = drip.popleft()
                    th()
                    spent_total += cost
                # release xin at step 96 (xr/wk/wv dead) and queue the
                # full Wo prefetch into the freed SBUF.
                if s == 96:
                    xin_cm.__exit__(None, None, None)
                    wo2_box.append(es.enter_context(
                        tc.tile_pool(name="wo2", bufs=1)))
                    for nt in range(4):
                        for i, (cst, th) in enumerate(gen_wo_prefetch(nt)):
                            drip.append((98 + nt * 6 + i // 3, cst, th))

            while inflight:
                emit_pv(*inflight.popleft())
            while drip:
                drip.popleft()[2]()

        # =============== phase E: output projection + bias ===========
        # sps/dps closed; ops stays open so pair 15's norm drain (which
        # reads po(15)) can overlap E's first 14 kc-groups — attnT[13]
        # and attnT[15] are the only pair-15-gated contraction chunks,
        # so they accumulate last.
        with tc.tile_pool(name="yps", bufs=6, space="PSUM") as yps, \
             tc.tile_pool(name="ystg", bufs=4) as ystg:
            bc_box[0] = (yps, "py")
            kc_order = list(range(13)) + [14, 13, 15]
            for nt in range(4):        # 4 output column blocks of 512
                wo_last = wo_pre[(nt, "last")]
                pys = [yps.tile([128, 512], F32, tag="py",
                                name=f"py{nt}_{i}") for i in range(4)]
                for idx, kc in enumerate(kc_order):
                    wo_t = wo_pre[(nt, kc)]
                    for mt in range(4):
                        nc.tensor.matmul(
                            out=pys[mt][:, :],
                            lhsT=attnT[kc][:, mt * 128:(mt + 1) * 128],
                            rhs=wo_t[:, :],
                            start=(idx == 0), stop=False)
                    if nt == 0 and idx == 13:
                        # pair-15 norm drain: DVE chain runs while the
                        # PE streams the kc-groups emitted above
                        for ci in (2, 4):
                            norm_copy(15, ci)
                        for ci in range(4, 16, 2):
                            norm_stage2(15, ci)
                for mt in range(4):    # bias via ones row, K=1 matmul
                    nc.tensor.matmul(
                        out=pys[mt][:, :], lhsT=ones128[:, :],
                        rhs=wo_last[:, :], start=False, stop=True)
                    ys = ystg.tile([128, 512], F32, tag="ys",
                                   name=f"ys{nt}_{mt}")
                    nc.vector.tensor_copy(out=ys[:, :], in_=pys[mt][:, :])
                    nc.sync.dma_start(
                        out=out[mt * 128:(mt + 1) * 128,
                                nt * 512:(nt + 1) * 512],
                        in_=ys[:, :])
        ops_cm.__exit__(None, None, None)

    nc.finalize()
    return nc


@functools.lru_cache(maxsize=1)
def _graph():
    return build_graph()


def make_in_maps(x, Wq, Wk, Wv, Wo, bo):
    bf16 = ml_dtypes.bfloat16
    x = np.asarray(x, np.float32)
    wqT = np.ascontiguousarray(np.asarray(Wq, np.float32).T).astype(bf16)
    wkT = np.ascontiguousarray(np.asarray(Wk, np.float32).T).astype(bf16)
    wvT = np.ascontiguousarray(np.asarray(Wv, np.float32).T).astype(bf16)
    woT = np.concatenate(
        [np.asarray(Wo, np.float32).T,
         np.asarray(bo, np.float32)[None, :]], axis=0).astype(bf16)
    woT = np.ascontiguousarray(woT)
    in_maps = []
    for c in range(8):
        b, r = c // TP, c % TP
        # token permutation: own query block first, rest after (attention
        # is permutation-invariant over keys)
        perm = np.r_[r * TOK:(r + 1) * TOK, 0:r * TOK, (r + 1) * TOK:S]
        xT_c = np.ascontiguousarray(x[b].T[:, perm]).astype(bf16)
        in_maps.append(
            {"xT": xT_c, "wqT": wqT, "wkT": wkT, "wvT": wvT, "woT": woT})
    return in_maps


def kernel(x, Wq, Wk, Wv, Wo, bo):
    nc = _graph()
    in_maps = make_in_maps(x, Wq, Wk, Wv, Wo, bo)
    res = run_bass_kernel_spmd(nc, in_maps, core_ids=list(range(8)))
    out = np.empty((B, S, HID), np.float32)
    for c in range(8):
        b, r = c // TP, c % TP
        out[b, r * TOK:(r + 1) * TOK, :] = np.asarray(
            res.results[c]["out"], np.float32)
    return out


# revision 26
# speedup vs baseline: 1.0081x; 1.0081x over previous
"""GQA attention (B=2, S=2048, HID=2048, 32 q heads / 8 kv heads, fp32 I/O)
on 8 TRN2 NeuronCores.

Sharding: sequence-parallel with fully local K/V. Core c owns 512 query
tokens of batch c//4 (cores 0-3 = batch 0, cores 4-7 = batch 1), but
computes K^T and V for ALL 2048 tokens of its batch locally — that
(+~40% KV projection FLOPs) is much cheaper than an intra-chip
AllGather, which measures 100-170us and blockades the DMA engines while
it runs. Attention is permutation-invariant over keys, so each core
orders tokens own-block-first (host-side permutation) and the device
program stays rank-independent.

V carries a fused ones-column per kv head so the PV matmul also
produces the softmax row-sums; the output-projection bias is fused as
an extra contraction row. All matmuls run in bf16 with fp32 PSUM
accumulation. Score matmuls (K=64) pair head A (partitions 0-63) and
head B (64-127) on disjoint PE row-groups so the hardware co-executes
them (auto tile_position from base partitions).

v2 schedule: the serial K/V-projection prologue is collapsed to one
8-bank PSUM wave {K mt0 x4, Q0, Q2, V(c0,kv0-3), V(c1,kv0-3)} streamed
chunk-outer against the x DMA arrival, so the first exp fires at
~45us (was ~122us). Everything else — V in 30 finer (chunk, kv-half)
units, K mt1-mt3, Q chunks, Wo prefetch — drips into the ACT-bound
pair loop through a 2-bank ping-pong PSUM ring with deadline-forced,
cost-budgeted pops. The xin pool (x remainder + Wk/Wv) releases at
step 96 to make room for prefetching all 4 Wo column blocks, so the
output projection runs as a pure-PE tail with all 8 PSUM banks.
"""

import functools
from collections import deque
from contextlib import ExitStack

import numpy as np
import ml_dtypes

import concourse.bass as bass
import concourse.mybir as mybir
import concourse.tile as tile
from concourse import bacc
from concourse.bass_utils import run_bass_kernel_spmd

BF = mybir.dt.bfloat16
F32 = mybir.dt.float32

B, S, HID = 2, 2048, 2048
NH, NKV, HD = 32, 8, 64          # q heads, kv heads, head dim
GRP = NH // NKV                  # 4 q heads per kv head
TP = 4                           # cores per batch group
TOK = S // TP                    # 512 local query tokens per core
KC = HID // 128                  # 16 contraction chunks of 128
NKC = S // 128                   # 16 key chunks of 128 (full seq)
VW = NKV * (HD + 1)              # 520: V width incl. ones columns
EXP_SCALE = float(HD) ** -0.5    # 1/8 softmax scale, fused into Exp
LAG = 2                          # steps between scores+exp and its PV


def q_slot(h):
    """qTp tile index and partition base for head h.

    Head h lives at partition base ((h//4)%2)*64 — the same base its kv
    head kh=h//4 occupies inside the kTg tiles, so the scores matmul's
    lhsT and rhs stay partition-aligned (and heads A/B co-execute on
    disjoint PE row groups).
    """
    return ((h // 4) // 2) * 4 + (h % 4), ((h // 4) % 2) * 64


def build_graph():
    nc = bacc.Bacc(None, target_bir_lowering=False, debug=False, num_devices=8)

    # DMA issue slots on the sync queue cost ~650ns EACH regardless of
    # size, so inputs are host-packed for one-issue-per-tile transfers:
    # wkvT = Wk^T|Wv^T fused, wqTk = Wq^T pre-tiled so a whole Q-chunk's
    # 16 weight tiles land in one [128, 16*128] DMA.
    xT = nc.declare_dram_parameter("xT", [HID, S], BF, isOutput=False)
    wkvT = nc.declare_dram_parameter("wkvT", [HID, 2 * NKV * HD], BF,
                                     isOutput=False)
    wqTk = nc.declare_dram_parameter("wqTk", [128, KC, HID], BF,
                                     isOutput=False)
    woT = nc.declare_dram_parameter("woT", [HID + 1, HID], BF, isOutput=False)
    out = nc.declare_dram_parameter("out", [TOK, HID], F32, isOutput=True)

    with tile.TileContext(nc) as tc, ExitStack() as es:
        pers = es.enter_context(tc.tile_pool(name="pers", bufs=1))

        def T(shape, dtype, *, name):
            return pers.tile(shape, dtype, name=name, tag=name)

        # long-lived SBUF pools first; xin LAST so it can release at
        # step 96 while still top-of-stack among SBUF pools.
        wqp = es.enter_context(tc.tile_pool(name="wqp", bufs=4))
        stgB = es.enter_context(tc.tile_pool(name="stgB", bufs=2))
        nrm = es.enter_context(tc.tile_pool(name="nrm", bufs=2))
        ptp = es.enter_context(tc.tile_pool(name="ptp", bufs=6))

        xin_cm = tc.tile_pool(name="xin", bufs=1)
        xin = xin_cm.__enter__()
        xf = [xin.tile([128, S], BF, tag=f"xf{k}", name=f"xf{k}")
              for k in range(KC)]
        wkv = [xin.tile([128, 2 * NKV * HD], BF, tag=f"wkv{k}", name=f"wkv{k}")
               for k in range(KC)]

        def wk_col(k, lo, n):
            return wkv[k][:, lo:lo + n]

        def wv_col(k, lo, n):
            return wkv[k][:, 512 + lo:512 + lo + n]

        # whole-unit weight tiles for wave-0's Q0/Q2
        wq_w0 = {m: wqp.tile([128, KC * 128], BF, tag="wq", name=f"wqw{m}")
                 for m in (0, 2)}

        # ---- DMA issue order = priority. Two issues per x chunk; wq
        # whole-unit tiles first so wave 0's Q matmuls never stall the
        # in-order PE queue.
        for m in (0, 2):
            nc.sync.dma_start(out=wq_w0[m][:, :],
                              in_=wqTk[:, :, m * 128:(m + 1) * 128])
        for k in range(KC):
            nc.sync.dma_start(out=wkv[k][:, :],
                              in_=wkvT[k * 128:(k + 1) * 128, :])
            nc.sync.dma_start(out=xf[k][:, :], in_=xT[k * 128:(k + 1) * 128, :])

        def xcols(k, lo, n):
            # columns lo..lo+n of the permuted x^T chunk k
            return xf[k][:, lo:lo + n]

        # row HD (partition 64) is the K=1 lhsT for the row-sum broadcast
        ones64 = T([HD + 1, 64], BF, name="ones64")
        nc.vector.memset(ones64[:, :], 1.0)
        ones128 = T([1, 128], BF, name="ones128")
        nc.vector.memset(ones128[:, :], 1.0)

        # kTg[nb*4+mt]: [128, 512] = K^T rows mt*128.. for key block nb
        # (kv heads 2mt at partitions 0-63, 2mt+1 at 64-127).
        # vg[c]: [128, 520] V_aug rows for key chunk c, ones at col
        # kh*65+64 of each kv head kh.
        kTg = [T([128, TOK], BF, name=f"kTg{i}") for i in range(16)]
        vg = [T([128, VW], BF, name=f"vg{c}") for c in range(NKC)]
        qTp = [T([128, TOK], BF, name=f"qTp{i}") for i in range(NH // 2)]
        attnT = [T([128, TOK], BF, name=f"attnT{t}") for t in range(NH // 2)]

        def q_fin(ps, m):
            st = stgB.tile([128, TOK], BF, tag="stg", name=f"stq{m}")
            nc.vector.tensor_copy(out=st[:, :], in_=ps[:, :])
            for j in range(2):
                h = 2 * m + j
                i, roff = q_slot(h)
                nc.sync.dma_start(out=qTp[i][roff:roff + 64, :],
                                  in_=st[j * 64:(j + 1) * 64, :])

        def v_fin(ps, c, h):
            if h == 0:
                nc.vector.memset(vg[c][:, :], 1.0)
            for kh in range(4 * h, 4 * h + 4):
                nc.vector.tensor_copy(
                    out=vg[c][:, kh * (HD + 1):kh * (HD + 1) + HD],
                    in_=ps[:, (kh - 4 * h) * 64:(kh - 4 * h + 1) * 64])

        # =============== wave 0: the minimal exp-gating work ============
        # 8 PSUM accumulation groups, contraction-chunk OUTER so the PE
        # streams 8 matmuls per arriving x chunk. Completes ~1.7us after
        # the last x chunk lands; first exp fires ~2us later.
        w0 = ([("k", 0, nb) for nb in range(TP)]
              + [("q", 0, None), ("q", 2, None)]
              + [("v", 0, 0), ("v", 1, 0)])
        with tc.tile_pool(name="accA", bufs=8, space="PSUM") as accA:
            pss = [accA.tile([128, TOK], F32, tag="acc", name=f"psA{i}")
                   for i in range(8)]
            for k in range(KC):
                for ps, (kind, a, b) in zip(pss, w0):
                    if kind == "k":
                        nc.tensor.matmul(
                            out=ps[:, :],
                            lhsT=wk_col(k, 0, 128),
                            rhs=xcols(k, b * TOK, TOK),
                            start=(k == 0), stop=(k == KC - 1))
                    elif kind == "q":
                        nc.tensor.matmul(
                            out=ps[:, :],
                            lhsT=wq_w0[a][:, k * 128:(k + 1) * 128],
                            rhs=xcols(k, 0, TOK),
                            start=(k == 0), stop=(k == KC - 1))
                    else:
                        nc.tensor.matmul(
                            out=ps[:, 0:256],
                            lhsT=xcols(k, a * 128, 128),
                            rhs=wv_col(k, 0, 256),
                            start=(k == 0), stop=(k == KC - 1))
            # evac in exp-gating order: kTg nb0, Q0, Q2, then the rest
            nc.vector.tensor_copy(out=kTg[0][:, :], in_=pss[0][:, :])
            q_fin(pss[4], 0)
            q_fin(pss[5], 2)
            for nb in range(1, TP):
                nc.vector.tensor_copy(out=kTg[nb * 4][:, :],
                                      in_=pss[nb][:, :])
            v_fin(pss[6][:, 0:256], 0, 0)
            v_fin(pss[7][:, 0:256], 1, 0)

        # =============== drip units (fed into the pair loop) ============
        # Each unit: (deadline_step, [(cost_ns, thunk), ...]).
        MM_NS = 216    # 512-col bf16 matmul streaming time
        VMM_NS = 112   # 256-col

        drip = deque()  # (deadline, cost_ns, thunk)
        _units = []     # (deadline, seq, [(cost, thunk), ...])

        def push_unit(deadline, cts):
            _units.append((deadline, len(_units), cts))

        def seal_units():
            # stable-sort by real deadline, then tighten each deadline to
            # a uniform ~1.1us/step spread: pops stay small (the in-order
            # PE queue must never bury the next scores under a drip
            # burst) and the queue cannot run dry early (an idle PE drops
            # to the mid p-state and the whole step chain slows ~60%).
            cum = 0
            for d, _, cts in sorted(_units, key=lambda u: (u[0], u[1])):
                ucost = sum(c for c, _ in cts)
                cum += ucost
                d_eff = min(d, max(0, int(cum / 1100) - 2))
                for c, t in cts:
                    drip.append((d_eff, c, t))

        # unit PSUM tiles MUST allocate lazily at pop time: the dps ring
        # orders its WAR handoffs by .tile() call order, which has to
        # match emission order (norm_bcast's pb tiles share the ring).
        def gen_q_chunk(m):
            """one whole-unit weight DMA, 16 matmuls, finisher."""
            box = {}
            ws = wqp.tile([128, KC * 128], BF, tag="wq", name=f"wqu{m}")
            def dm(m=m):
                nc.sync.dma_start(out=ws[:, :],
                                  in_=wqTk[:, :, m * 128:(m + 1) * 128])
            thunks = [(0, dm)]
            for k in range(KC):
                def mm(k=k, m=m):
                    if k == 0:
                        box["ps"] = dps.tile([128, TOK], F32, tag="dp",
                                             name=f"psq{m}")
                    nc.tensor.matmul(
                        out=box["ps"][:, :],
                        lhsT=ws[:, k * 128:(k + 1) * 128],
                        rhs=xcols(k, 0, TOK),
                        start=(k == 0), stop=(k == KC - 1))
                thunks.append((MM_NS, mm))
            thunks.append((0, lambda m=m: q_fin(box["ps"], m)))
            return thunks

        def gen_k_unit(mt, nb):
            box = {}
            thunks = []
            for k in range(KC):
                def mm(k=k, mt=mt, nb=nb):
                    if k == 0:
                        box["ps"] = dps.tile([128, TOK], F32, tag="dp",
                                             name=f"psk{nb}_{mt}")
                    nc.tensor.matmul(
                        out=box["ps"][:, :],
                        lhsT=wk_col(k, mt * 128, 128),
                        rhs=xcols(k, nb * TOK, TOK),
                        start=(k == 0), stop=(k == KC - 1))
                thunks.append((MM_NS, mm))
            def fin(mt=mt, nb=nb):
                nc.vector.tensor_copy(out=kTg[nb * 4 + mt][:, :],
                                      in_=box["ps"][:, :])
            thunks.append((0, fin))
            return thunks

        def gen_v_unit(c, h):
            box = {}
            thunks = []
            for k in range(KC):
                def mm(k=k, c=c, h=h):
                    if k == 0:
                        box["ps"] = dps.tile([128, TOK], F32, tag="dp",
                                             name=f"psv{c}_{h}")
                    nc.tensor.matmul(
                        out=box["ps"][:, 0:256],
                        lhsT=xcols(k, c * 128, 128),
                        rhs=wv_col(k, h * 256, 256),
                        start=(k == 0), stop=(k == KC - 1))
                thunks.append((VMM_NS, mm))
            thunks.append(
                (0, lambda c=c, h=h: v_fin(box["ps"][:, 0:256], c, h)))
            return thunks

        # Wo prefetch (DMA-only): queued when xin releases at step 118.
        # One [128, 2048] row-block DMA per kc (all 4 nt at once).
        wo_pre = {}
        wo2_box = []

        def gen_wo_prefetch():
            thunks = []
            def last():
                wl = wo2_box[0].tile([1, HID], BF, tag="wolast",
                                     bufs=1, name="wl")
                nc.sync.dma_start(out=wl[:, :], in_=woT[HID:HID + 1, :])
                for nt in range(4):
                    wo_pre[(nt, "last")] = wl[:, nt * 512:(nt + 1) * 512]
            thunks.append((0, last))
            for kc in range(KC):
                def f(kc=kc):
                    w = wo2_box[0].tile([128, HID], BF, tag="wo", bufs=16,
                                        name=f"wo{kc}")
                    nc.sync.dma_start(
                        out=w[:, :], in_=woT[kc * 128:(kc + 1) * 128, :])
                    for nt in range(4):
                        wo_pre[(nt, kc)] = w[:, nt * 512:(nt + 1) * 512]
                thunks.append((0, f))
            return thunks

        # =============== the pair loop ==================================
        pairs = []
        for g in range(0, NKV, 2):
            for j in range(GRP):
                pairs.append((g * GRP + j, (g + 1) * GRP + j))

        bc_box = [None]  # (pool, tag) for the row-sum broadcast PSUM
        ops_cm = tc.tile_pool(name="ops", bufs=2, space="PSUM")
        ops = ops_cm.__enter__()
        with tc.tile_pool(name="sps", bufs=2, space="PSUM") as sps, \
             tc.tile_pool(name="dps", bufs=2, space="PSUM") as dps:
            bc_box[0] = (dps, "dp")

            # drip queue in deadline order
            def push_q(d, m):
                ths = gen_q_chunk(m)
                # weight DMA leads its matmuls by ~4 steps so the 512KB
                # transfer never head-of-line blocks the PE queue
                push_unit(max(0, d - 4), [ths[0]])
                push_unit(d, ths[1:])

            for c in range(2, NKC):                      # V kv0-3 rest
                push_unit(max(0, c // 2 - 1), gen_v_unit(c, 0))
            push_q(13, 1)
            push_q(13, 3)
            for nb in range(TP):                         # K mt1
                push_unit(28 + 2 * nb, gen_k_unit(1, nb))
            push_q(29, 4)
            push_q(29, 6)
            push_q(44, 5)
            push_q(44, 7)
            for c in range(NKC):                         # V kv4-7
                push_unit(61 + c // 2, gen_v_unit(c, 1))
            for nb in range(TP):                         # K mt2
                push_unit(61 + 2 * nb, gen_k_unit(2, nb))
            push_q(61, 8)
            push_q(61, 10)
            push_q(76, 9)
            push_q(76, 11)
            for nb in range(TP):                         # K mt3
                push_unit(92 + 2 * nb, gen_k_unit(3, nb))
            push_q(93, 12)
            push_q(93, 14)
            push_q(105, 13)
            push_q(105, 15)
            seal_units()

            # Normalization for pair p staged across pair p+1's steps
            # (PSUM->SBUF copy, reciprocal, cast, PE ones-broadcast via
            # the dps ring, multiply) so the 3.3us DVE reciprocal never
            # blocks a PE-side consumer.
            aph_of = {}

            def norm_copy(p, ci):
                j = 0 if ci == 2 else 1
                h = pairs[p][j]
                po = po_of[p][j]
                aph = nrm.tile([HD + 1, TOK], F32, tag="aph", bufs=2,
                               name=f"aph{h}")
                nc.vector.tensor_copy(out=aph[:, :], in_=po[:, :])
                aph_of[(p, j)] = [aph, None, None]
                if ci == 4:
                    del po_of[p]

            def norm_recip(p, j):
                h = pairs[p][j]
                ent = aph_of[(p, j)]
                rcp = nrm.tile([HD + 1, TOK], F32, tag="rcp", name=f"rc{h}")
                nc.vector.reciprocal(out=rcp[HD:HD + 1, :],
                                     in_=ent[0][HD:HD + 1, :])
                ent[1] = rcp

            def norm_cast(p, j):
                h = pairs[p][j]
                ent = aph_of[(p, j)]
                rcpb = nrm.tile([HD + 1, TOK], BF, tag="rcpb", name=f"rb{h}")
                nc.vector.tensor_copy(out=rcpb[HD:HD + 1, :],
                                      in_=ent[1][HD:HD + 1, :])
                ent[1] = rcpb

            def norm_bcast(p, j):
                h = pairs[p][j]
                ent = aph_of[(p, j)]
                pool, tag = bc_box[0]
                pb = pool.tile([128, TOK], F32, tag=tag, name=f"pb{h}")
                nc.tensor.matmul(out=pb[0:64, :], lhsT=ones64[HD:HD + 1, :],
                                 rhs=ent[1][HD:HD + 1, :],
                                 start=True, stop=True)
                rb = nrm.tile([64, TOK], BF, tag="rbb", name=f"rbb{h}")
                nc.vector.tensor_copy(out=rb[:, :], in_=pb[0:64, :])
                ent[2] = rb

            def norm_mul(p, j):
                h = pairs[p][j]
                ent = aph_of[(p, j)]
                t, half = h // 2, (h % 2) * 64
                if half == 0:
                    nc.vector.tensor_mul(out=attnT[t][0:64, :],
                                         in0=ent[0][0:HD, :],
                                         in1=ent[2][:, :])
                else:
                    ah = nrm.tile([64, TOK], BF, tag="ah", name=f"ah{h}")
                    nc.vector.tensor_mul(out=ah[:, :], in0=ent[0][0:HD, :],
                                         in1=ent[2][:, :])
                    nc.sync.dma_start(out=attnT[t][64:128, :], in_=ah[:, :])
                del aph_of[(p, j)]

            def norm_stage2(p, ci):
                if ci == 4:
                    norm_recip(p, 0)
                elif ci == 6:
                    norm_recip(p, 1)
                elif ci == 8:
                    norm_cast(p, 0)
                    norm_cast(p, 1)
                elif ci == 10:
                    norm_bcast(p, 0)
                elif ci == 12:
                    norm_bcast(p, 1)
                    norm_mul(p, 0)
                elif ci == 14:
                    norm_mul(p, 1)

            def emit_scores(pi, ci):
                hA, hB = pairs[pi]
                kt = (hA // GRP) // 2
                qiA, _ = q_slot(hA)
                qiB, _ = q_slot(hB)
                psA = sps.tile([128, 2 * TOK], F32, tag="ps",
                               name=f"psA{hA}_{ci}")
                psB = sps.tile([128, 2 * TOK], F32, tag="ps",
                               name=f"psB{hB}_{ci}")
                for dc in range(2):
                    c = ci + dc
                    nb, lc = c // 4, c % 4
                    kts = kTg[nb * 4 + kt]
                    nc.tensor.matmul(
                        out=psA[:, dc * TOK:(dc + 1) * TOK],
                        lhsT=kts[0:64, lc * 128:(lc + 1) * 128],
                        rhs=qTp[qiA][0:64, :], start=True, stop=True)
                ptA = ptp.tile([128, 2 * TOK], BF, tag="pt",
                               name=f"ptA{hA}_{ci}")
                nc.scalar.activation(
                    out=ptA[:, :], in_=psA[:, :],
                    func=mybir.ActivationFunctionType.Exp, scale=EXP_SCALE)
                for dc in range(2):
                    c = ci + dc
                    nb, lc = c // 4, c % 4
                    kts = kTg[nb * 4 + kt]
                    nc.tensor.matmul(
                        out=psB[:, dc * TOK:(dc + 1) * TOK],
                        lhsT=kts[64:128, lc * 128:(lc + 1) * 128],
                        rhs=qTp[qiB][64:128, :], start=True, stop=True)
                ptB = ptp.tile([128, 2 * TOK], BF, tag="pt",
                               name=f"ptB{hB}_{ci}")
                nc.scalar.activation(
                    out=ptB[:, :], in_=psB[:, :],
                    func=mybir.ActivationFunctionType.Exp, scale=EXP_SCALE)
                return ptA, ptB

            def emit_pv(pi, ci, ptA, ptB):
                hA, hB = pairs[pi]
                khA, khB = hA // GRP, hB // GRP
                poA, poB = po_of[pi]
                for dc in range(2):
                    c = ci + dc
                    nc.tensor.matmul(
                        out=poA[:, :],
                        lhsT=vg[c][:, khA * (HD + 1):(khA + 1) * (HD + 1)],
                        rhs=ptA[:, dc * TOK:(dc + 1) * TOK],
                        start=(c == 0), stop=(c == NKC - 1))
                    nc.tensor.matmul(
                        out=poB[:, :],
                        lhsT=vg[c][:, khB * (HD + 1):(khB + 1) * (HD + 1)],
                        rhs=ptB[:, dc * TOK:(dc + 1) * TOK],
                        start=(c == 0), stop=(c == NKC - 1))

            sched = [(pi, 2 * c2) for pi in range(len(pairs))
                     for c2 in range(NKC // 2)]
            po_of = {}
            inflight = deque()

            for s, (pi, ci) in enumerate(sched):
                hA, hB = pairs[pi]
                if ci == 0:
                    poA = ops.tile([HD + 1, TOK], F32, tag="po",
                                   name=f"poA{hA}")
                    poB = ops.tile([HD + 1, TOK], F32, tag="po",
                                   name=f"poB{hB}")
                    po_of[pi] = (poA, poB)
                # PV of step s-LAG first: it never waits, so it fills the
                # window where scores-A(s) stalls on exp(s-1) freeing the
                # score-PSUM ring (the PE queue is in-order).
                if len(inflight) >= LAG:
                    emit_pv(*inflight.popleft())
                ptA, ptB = emit_scores(pi, ci)
                inflight.append((pi, ci, ptA, ptB))
                if pi > 0:
                    if ci in (2, 4):
                        norm_copy(pi - 1, ci)
                    if ci >= 4:
                        norm_stage2(pi - 1, ci)
                # purely deadline-driven drip pops (deadlines carry the
                # uniform pacing from seal_units)
                while drip and drip[0][0] <= s + 2:
                    _, cost, th = drip.popleft()
                    th()
                # release xin at step 118 (all x/wk/wv consumers done by
                # ~110) and stream the Wo prefetch into the freed SBUF.
                if s == 118:
                    xin_cm.__exit__(None, None, None)
                    wo2_box.append(es.enter_context(
                        tc.tile_pool(name="wo2", bufs=1)))
                    for i, (cst, th) in enumerate(gen_wo_prefetch()):
                        drip.append((119 + i, cst, th))

            while inflight:
                emit_pv(*inflight.popleft())
            while drip:
                drip.popleft()[2]()

        # =============== phase E: output projection + bias ===========
        # sps/dps closed; ops stays open so pair 15's norm drain (which
        # reads po(15)) can overlap E's first 14 kc-groups — attnT[13]
        # and attnT[15] are the only pair-15-gated contraction chunks,
        # so they accumulate last.
        with tc.tile_pool(name="yps", bufs=6, space="PSUM") as yps, \
             tc.tile_pool(name="ystg", bufs=4) as ystg:
            bc_box[0] = (yps, "py")
            kc_order = list(range(13)) + [14, 13, 15]
            for nt in range(4):        # 4 output column blocks of 512
                wo_last = wo_pre[(nt, "last")]
                pys = [yps.tile([128, 512], F32, tag="py",
                                name=f"py{nt}_{i}") for i in range(4)]
                for idx, kc in enumerate(kc_order):
                    wo_t = wo_pre[(nt, kc)]
                    for mt in range(4):
                        nc.tensor.matmul(
                            out=pys[mt][:, :],
                            lhsT=attnT[kc][:, mt * 128:(mt + 1) * 128],
                            rhs=wo_t[:, :],
                            start=(idx == 0), stop=False)
                    if nt == 0 and idx == 13:
                        # pair-15 norm drain: DVE chain runs while the
                        # PE streams the kc-groups emitted above
                        for ci in (2, 4):
                            norm_copy(15, ci)
                        for ci in range(4, 16, 2):
                            norm_stage2(15, ci)
                for mt in range(4):    # bias via ones row, K=1 matmul
                    nc.tensor.matmul(
                        out=pys[mt][:, :], lhsT=ones128[:, :],
                        rhs=wo_last[:, :], start=False, stop=True)
                    ys = ystg.tile([128, 512], F32, tag="ys",
                                   name=f"ys{nt}_{mt}")
                    nc.vector.tensor_copy(out=ys[:, :], in_=pys[mt][:, :])
                    nc.sync.dma_start(
                        out=out[mt * 128:(mt + 1) * 128,
                                nt * 512:(nt + 1) * 512],
                        in_=ys[:, :])
        ops_cm.__exit__(None, None, None)

    nc.finalize()
    return nc


@functools.lru_cache(maxsize=1)
def _graph():
    return build_graph()


def make_in_maps(x, Wq, Wk, Wv, Wo, bo):
    bf16 = ml_dtypes.bfloat16
    x = np.asarray(x, np.float32)
    wqT = np.asarray(Wq, np.float32).T                    # [HID, HID]
    # pre-tiled so one [128, KC*128] DMA loads a whole Q chunk's weights
    wqTk = np.ascontiguousarray(
        wqT.reshape(KC, 128, HID).transpose(1, 0, 2)).astype(bf16)
    wkvT = np.ascontiguousarray(np.concatenate(
        [np.asarray(Wk, np.float32).T, np.asarray(Wv, np.float32).T],
        axis=1)).astype(bf16)                             # [HID, 1024]
    woT = np.concatenate(
        [np.asarray(Wo, np.float32).T,
         np.asarray(bo, np.float32)[None, :]], axis=0).astype(bf16)
    woT = np.ascontiguousarray(woT)
    in_maps = []
    for c in range(8):
        b, r = c // TP, c % TP
        # token permutation: own query block first, rest after (attention
        # is permutation-invariant over keys)
        perm = np.r_[r * TOK:(r + 1) * TOK, 0:r * TOK, (r + 1) * TOK:S]
        xT_c = np.ascontiguousarray(x[b].T[:, perm]).astype(bf16)
        in_maps.append(
            {"xT": xT_c, "wkvT": wkvT, "wqTk": wqTk, "woT": woT})
    return in_maps


def kernel(x, Wq, Wk, Wv, Wo, bo):
    nc = _graph()
    in_maps = make_in_maps(x, Wq, Wk, Wv, Wo, bo)
    res = run_bass_kernel_spmd(nc, in_maps, core_ids=list(range(8)))
    out = np.empty((B, S, HID), np.float32)
    for c in range(8):
        b, r = c // TP, c % TP
        out[b, r * TOK:(r + 1) * TOK, :] = np.asarray(
            res.results[c]["out"], np.float32)
    return out


# revision 29
# speedup vs baseline: 1.0460x; 1.0375x over previous
"""GQA attention (B=2, S=2048, HID=2048, 32 q heads / 8 kv heads, fp32 I/O)
on 8 TRN2 NeuronCores.

Sharding: sequence-parallel with fully local K/V. Core c owns 512 query
tokens of batch c//4 (cores 0-3 = batch 0, cores 4-7 = batch 1), but
computes K^T and V for ALL 2048 tokens of its batch locally — that
(+~40% KV projection FLOPs) is much cheaper than an intra-chip
AllGather, which measures 100-170us and blockades the DMA engines while
it runs. Attention is permutation-invariant over keys, so each core
orders tokens own-block-first (host-side permutation) and the device
program stays rank-independent.

V carries a fused ones-column per kv head so the PV matmul also
produces the softmax row-sums; the output-projection bias is fused as
an extra contraction row. All matmuls run in bf16 with fp32 PSUM
accumulation. Score matmuls (K=64) pair head A (partitions 0-63) and
head B (64-127) on disjoint PE row-groups so the hardware co-executes
them (auto tile_position from base partitions).

v2 schedule: the serial K/V-projection prologue is collapsed to one
8-bank PSUM wave {K mt0 x4, Q0, Q2, V(c0,kv0-3), V(c1,kv0-3)} streamed
chunk-outer against the x DMA arrival, so the first exp fires at
~45us (was ~122us). Everything else — V in 30 finer (chunk, kv-half)
units, K mt1-mt3, Q chunks, Wo prefetch — drips into the ACT-bound
pair loop through a 2-bank ping-pong PSUM ring with deadline-forced,
cost-budgeted pops. The xin pool (x remainder + Wk/Wv) releases at
step 96 to make room for prefetching all 4 Wo column blocks, so the
output projection runs as a pure-PE tail with all 8 PSUM banks.
"""

import functools
from collections import deque
from contextlib import ExitStack

import numpy as np
import ml_dtypes

import concourse.bass as bass
import concourse.mybir as mybir
import concourse.tile as tile
from concourse import bacc
from concourse.bass_utils import run_bass_kernel_spmd

BF = mybir.dt.bfloat16
F32 = mybir.dt.float32

B, S, HID = 2, 2048, 2048
NH, NKV, HD = 32, 8, 64          # q heads, kv heads, head dim
GRP = NH // NKV                  # 4 q heads per kv head
TP = 4                           # cores per batch group
TOK = S // TP                    # 512 local query tokens per core
KC = HID // 128                  # 16 contraction chunks of 128
NKC = S // 128                   # 16 key chunks of 128 (full seq)
VW = NKV * (HD + 1)              # 520: V width incl. ones columns
EXP_SCALE = float(HD) ** -0.5    # 1/8 softmax scale, fused into Exp
LAG = 2                          # steps between scores+exp and its PV


def q_slot(h):
    """qTp tile index and partition base for head h.

    Head h lives at partition base ((h//4)%2)*64 — the same base its kv
    head kh=h//4 occupies inside the kTg tiles, so the scores matmul's
    lhsT and rhs stay partition-aligned (and heads A/B co-execute on
    disjoint PE row groups).
    """
    return ((h // 4) // 2) * 4 + (h % 4), ((h // 4) % 2) * 64


def build_graph():
    nc = bacc.Bacc(None, target_bir_lowering=False, debug=False, num_devices=8)

    # DMA issue slots on the sync queue cost ~650ns EACH regardless of
    # size, so inputs are host-packed for one-issue-per-tile transfers:
    # wkvT = Wk^T|Wv^T fused, wqTk = Wq^T pre-tiled so a whole Q-chunk's
    # 16 weight tiles land in one [128, 16*128] DMA.
    xT = nc.declare_dram_parameter("xT", [HID, S], BF, isOutput=False)
    wkvT = nc.declare_dram_parameter("wkvT", [HID, 2 * NKV * HD], BF,
                                     isOutput=False)
    wqTk = nc.declare_dram_parameter("wqTk", [128, KC, HID], BF,
                                     isOutput=False)
    woT = nc.declare_dram_parameter("woT", [HID + 1, HID], BF, isOutput=False)
    out = nc.declare_dram_parameter("out", [TOK, HID], F32, isOutput=True)

    with tile.TileContext(nc) as tc, ExitStack() as es:
        pers = es.enter_context(tc.tile_pool(name="pers", bufs=1))

        def T(shape, dtype, *, name):
            return pers.tile(shape, dtype, name=name, tag=name)

        # long-lived SBUF pools first; xin LAST so it can release at
        # step 96 while still top-of-stack among SBUF pools.
        wqp = es.enter_context(tc.tile_pool(name="wqp", bufs=4))
        stgB = es.enter_context(tc.tile_pool(name="stgB", bufs=2))
        nrm = es.enter_context(tc.tile_pool(name="nrm", bufs=2))
        ptp = es.enter_context(tc.tile_pool(name="ptp", bufs=6))

        xin_cm = tc.tile_pool(name="xin", bufs=1)
        xin = xin_cm.__enter__()
        xf = [xin.tile([128, S], BF, tag=f"xf{k}", name=f"xf{k}")
              for k in range(KC)]
        wkv = [xin.tile([128, 2 * NKV * HD], BF, tag=f"wkv{k}", name=f"wkv{k}")
               for k in range(KC)]

        def wk_col(k, lo, n):
            return wkv[k][:, lo:lo + n]

        def wv_col(k, lo, n):
            return wkv[k][:, 512 + lo:512 + lo + n]

        # whole-unit weight tiles for wave-0's Q0/Q2
        wq_w0 = {m: wqp.tile([128, KC * 128], BF, tag="wq", name=f"wqw{m}")
                 for m in (0, 2)}

        # ---- DMA issue order = priority. Two issues per x chunk; wq
        # whole-unit tiles first so wave 0's Q matmuls never stall the
        # in-order PE queue.
        for m in (0, 2):
            nc.sync.dma_start(out=wq_w0[m][:, :],
                              in_=wqTk[:, :, m * 128:(m + 1) * 128])
        for k in range(KC):
            nc.sync.dma_start(out=wkv[k][:, :],
                              in_=wkvT[k * 128:(k + 1) * 128, :])
            nc.sync.dma_start(out=xf[k][:, :], in_=xT[k * 128:(k + 1) * 128, :])

        def xcols(k, lo, n):
            # columns lo..lo+n of the permuted x^T chunk k
            return xf[k][:, lo:lo + n]

        # row HD (partition 64) is the K=1 lhsT for the row-sum broadcast
        ones64 = T([HD + 1, 64], BF, name="ones64")
        nc.vector.memset(ones64[:, :], 1.0)
        ones128 = T([1, 128], BF, name="ones128")
        nc.vector.memset(ones128[:, :], 1.0)

        # kTg[nb*4+mt]: [128, 512] = K^T rows mt*128.. for key block nb
        # (kv heads 2mt at partitions 0-63, 2mt+1 at 64-127).
        # vg[c]: [128, 520] V_aug rows for key chunk c, ones at col
        # kh*65+64 of each kv head kh.
        kTg = [T([128, TOK], BF, name=f"kTg{i}") for i in range(16)]
        vg = [T([128, VW], BF, name=f"vg{c}") for c in range(NKC)]
        qTp = [T([128, TOK], BF, name=f"qTp{i}") for i in range(NH // 2)]
        attnT = [T([128, TOK], BF, name=f"attnT{t}") for t in range(NH // 2)]

        def q_fin(ps, m):
            st = stgB.tile([128, TOK], BF, tag="stg", name=f"stq{m}")
            nc.vector.tensor_copy(out=st[:, :], in_=ps[:, :])
            for j in range(2):
                h = 2 * m + j
                i, roff = q_slot(h)
                nc.sync.dma_start(out=qTp[i][roff:roff + 64, :],
                                  in_=st[j * 64:(j + 1) * 64, :])

        def v_fin(ps, c, h):
            if h == 0:
                nc.vector.memset(vg[c][:, :], 1.0)
            for kh in range(4 * h, 4 * h + 4):
                nc.vector.tensor_copy(
                    out=vg[c][:, kh * (HD + 1):kh * (HD + 1) + HD],
                    in_=ps[:, (kh - 4 * h) * 64:(kh - 4 * h + 1) * 64])

        # =============== wave 0: the minimal exp-gating work ============
        # 8 PSUM accumulation groups, contraction-chunk OUTER so the PE
        # streams 8 matmuls per arriving x chunk. Completes ~1.7us after
        # the last x chunk lands; first exp fires ~2us later.
        w0 = ([("k", 0, nb) for nb in range(TP)]
              + [("q", 0, None), ("q", 2, None)]
              + [("v", 0, 0), ("v", 1, 0)])
        with tc.tile_pool(name="accA", bufs=8, space="PSUM") as accA:
            pss = [accA.tile([128, TOK], F32, tag="acc", name=f"psA{i}")
                   for i in range(8)]
            for k in range(KC):
                for ps, (kind, a, b) in zip(pss, w0):
                    if kind == "k":
                        nc.tensor.matmul(
                            out=ps[:, :],
                            lhsT=wk_col(k, 0, 128),
                            rhs=xcols(k, b * TOK, TOK),
                            start=(k == 0), stop=(k == KC - 1))
                    elif kind == "q":
                        nc.tensor.matmul(
                            out=ps[:, :],
                            lhsT=wq_w0[a][:, k * 128:(k + 1) * 128],
                            rhs=xcols(k, 0, TOK),
                            start=(k == 0), stop=(k == KC - 1))
                    else:
                        nc.tensor.matmul(
                            out=ps[:, 0:256],
                            lhsT=xcols(k, a * 128, 128),
                            rhs=wv_col(k, 0, 256),
                            start=(k == 0), stop=(k == KC - 1))
            # evac in exp-gating order: kTg nb0, Q0, Q2, then the rest
            nc.vector.tensor_copy(out=kTg[0][:, :], in_=pss[0][:, :])
            q_fin(pss[4], 0)
            q_fin(pss[5], 2)
            for nb in range(1, TP):
                nc.vector.tensor_copy(out=kTg[nb * 4][:, :],
                                      in_=pss[nb][:, :])
            v_fin(pss[6][:, 0:256], 0, 0)
            v_fin(pss[7][:, 0:256], 1, 0)

        # =============== drip units (fed into the pair loop) ============
        # Each unit: (deadline_step, [(cost_ns, thunk), ...]).
        MM_NS = 216    # 512-col bf16 matmul streaming time
        VMM_NS = 112   # 256-col

        drip = deque()  # (deadline, cost_ns, thunk)
        _units = []     # (deadline, seq, [(cost, thunk), ...])

        def push_unit(deadline, cts):
            _units.append((deadline, len(_units), cts))

        def seal_units():
            # stable-sort by real deadline, then tighten each deadline to
            # a uniform ~1.1us/step spread: pops stay small (the in-order
            # PE queue must never bury the next scores under a drip
            # burst) and the queue cannot run dry early (an idle PE drops
            # to the mid p-state and the whole step chain slows ~60%).
            cum = 0
            for d, _, cts in sorted(_units, key=lambda u: (u[0], u[1])):
                for c, t in cts:
                    cum += c
                    # per-THUNK spread: pops arrive in ~1.1us/step
                    # trickles, never as whole-unit bursts
                    d_eff = min(d, max(0, int(cum / 1100) - 2))
                    drip.append((d_eff, c, t))

        # unit PSUM tiles MUST allocate lazily at pop time: the dps ring
        # orders its WAR handoffs by .tile() call order, which has to
        # match emission order (norm_bcast's pb tiles share the ring).
        def gen_q_chunk(m):
            """one whole-unit weight DMA, 16 matmuls, finisher."""
            box = {}
            ws = wqp.tile([128, KC * 128], BF, tag="wq", name=f"wqu{m}")
            def dm(m=m):
                nc.sync.dma_start(out=ws[:, :],
                                  in_=wqTk[:, :, m * 128:(m + 1) * 128])
            thunks = [(0, dm)]
            for k in range(KC):
                def mm(k=k, m=m):
                    if k == 0:
                        box["ps"] = dps.tile([128, TOK], F32, tag="dp",
                                             name=f"psq{m}")
                    nc.tensor.matmul(
                        out=box["ps"][:, :],
                        lhsT=ws[:, k * 128:(k + 1) * 128],
                        rhs=xcols(k, 0, TOK),
                        start=(k == 0), stop=(k == KC - 1))
                thunks.append((MM_NS, mm))
            thunks.append((0, lambda m=m: q_fin(box["ps"], m)))
            return thunks

        def gen_k_unit(mt, nb):
            box = {}
            thunks = []
            for k in range(KC):
                def mm(k=k, mt=mt, nb=nb):
                    if k == 0:
                        box["ps"] = dps.tile([128, TOK], F32, tag="dp",
                                             name=f"psk{nb}_{mt}")
                    nc.tensor.matmul(
                        out=box["ps"][:, :],
                        lhsT=wk_col(k, mt * 128, 128),
                        rhs=xcols(k, nb * TOK, TOK),
                        start=(k == 0), stop=(k == KC - 1))
                thunks.append((MM_NS, mm))
            def fin(mt=mt, nb=nb):
                nc.vector.tensor_copy(out=kTg[nb * 4 + mt][:, :],
                                      in_=box["ps"][:, :])
            thunks.append((0, fin))
            return thunks

        def gen_v_unit(c, h):
            box = {}
            thunks = []
            for k in range(KC):
                def mm(k=k, c=c, h=h):
                    if k == 0:
                        box["ps"] = dps.tile([128, TOK], F32, tag="dp",
                                             name=f"psv{c}_{h}")
                    nc.tensor.matmul(
                        out=box["ps"][:, 0:256],
                        lhsT=xcols(k, c * 128, 128),
                        rhs=wv_col(k, h * 256, 256),
                        start=(k == 0), stop=(k == KC - 1))
                thunks.append((VMM_NS, mm))
            thunks.append(
                (0, lambda c=c, h=h: v_fin(box["ps"][:, 0:256], c, h)))
            return thunks

        # Wo prefetch (DMA-only): queued when xin releases at step 118.
        # One [128, 2048] row-block DMA per kc (all 4 nt at once).
        wo_pre = {}
        wo2_box = []

        def gen_wo_prefetch():
            thunks = []
            def last():
                wl = wo2_box[0].tile([1, HID], BF, tag="wolast",
                                     bufs=1, name="wl")
                nc.sync.dma_start(out=wl[:, :], in_=woT[HID:HID + 1, :])
                for nt in range(4):
                    wo_pre[(nt, "last")] = wl[:, nt * 512:(nt + 1) * 512]
            thunks.append((0, last))
            for kc in range(KC):
                def f(kc=kc):
                    w = wo2_box[0].tile([128, HID], BF, tag="wo", bufs=16,
                                        name=f"wo{kc}")
                    nc.sync.dma_start(
                        out=w[:, :], in_=woT[kc * 128:(kc + 1) * 128, :])
                    for nt in range(4):
                        wo_pre[(nt, kc)] = w[:, nt * 512:(nt + 1) * 512]
                thunks.append((0, f))
            return thunks

        # =============== the pair loop ==================================
        pairs = []
        for g in range(0, NKV, 2):
            for j in range(GRP):
                pairs.append((g * GRP + j, (g + 1) * GRP + j))

        bc_box = [None]  # (pool, tag) for the row-sum broadcast PSUM
        ops_cm = tc.tile_pool(name="ops", bufs=2, space="PSUM")
        ops = ops_cm.__enter__()
        with tc.tile_pool(name="sps", bufs=2, space="PSUM") as sps, \
             tc.tile_pool(name="dps", bufs=2, space="PSUM") as dps:
            bc_box[0] = (dps, "dp")

            # drip queue in deadline order
            def push_q(d, m):
                ths = gen_q_chunk(m)
                # weight DMA leads its matmuls by ~4 steps so the 512KB
                # transfer never head-of-line blocks the PE queue
                push_unit(max(0, d - 4), [ths[0]])
                push_unit(d, ths[1:])

            for c in range(2, NKC):                      # V kv0-3 rest
                push_unit(max(0, c // 2 - 1), gen_v_unit(c, 0))
            push_q(13, 1)
            push_q(13, 3)
            for nb in range(TP):                         # K mt1
                push_unit(28 + 2 * nb, gen_k_unit(1, nb))
            push_q(29, 4)
            push_q(29, 6)
            push_q(44, 5)
            push_q(44, 7)
            for c in range(NKC):                         # V kv4-7
                push_unit(61 + c // 2, gen_v_unit(c, 1))
            for nb in range(TP):                         # K mt2
                push_unit(61 + 2 * nb, gen_k_unit(2, nb))
            push_q(61, 8)
            push_q(61, 10)
            push_q(76, 9)
            push_q(76, 11)
            for nb in range(TP):                         # K mt3
                push_unit(92 + 2 * nb, gen_k_unit(3, nb))
            push_q(93, 12)
            push_q(93, 14)
            push_q(105, 13)
            push_q(105, 15)
            seal_units()

            # Normalization for pair p staged across pair p+1's steps
            # (PSUM->SBUF copy, reciprocal, cast, PE ones-broadcast via
            # the dps ring, multiply) so the 3.3us DVE reciprocal never
            # blocks a PE-side consumer.
            aph_of = {}

            def norm_copy(p, ci):
                j = 0 if ci == 2 else 1
                h = pairs[p][j]
                po = po_of[p][j]
                aph = nrm.tile([HD + 1, TOK], F32, tag="aph", bufs=2,
                               name=f"aph{h}")
                nc.vector.tensor_copy(out=aph[:, :], in_=po[:, :])
                aph_of[(p, j)] = [aph, None, None]
                if ci == 4:
                    del po_of[p]

            def norm_recip(p, j):
                h = pairs[p][j]
                ent = aph_of[(p, j)]
                rcp = nrm.tile([HD + 1, TOK], F32, tag="rcp", name=f"rc{h}")
                nc.vector.reciprocal(out=rcp[HD:HD + 1, :],
                                     in_=ent[0][HD:HD + 1, :])
                ent[1] = rcp

            def norm_cast(p, j):
                h = pairs[p][j]
                ent = aph_of[(p, j)]
                rcpb = nrm.tile([HD + 1, TOK], BF, tag="rcpb", name=f"rb{h}")
                nc.vector.tensor_copy(out=rcpb[HD:HD + 1, :],
                                      in_=ent[1][HD:HD + 1, :])
                ent[1] = rcpb

            def norm_bcast(p, j):
                h = pairs[p][j]
                ent = aph_of[(p, j)]
                pool, tag = bc_box[0]
                pb = pool.tile([128, TOK], F32, tag=tag, name=f"pb{h}")
                nc.tensor.matmul(out=pb[0:64, :], lhsT=ones64[HD:HD + 1, :],
                                 rhs=ent[1][HD:HD + 1, :],
                                 start=True, stop=True)
                rb = nrm.tile([64, TOK], BF, tag="rbb", name=f"rbb{h}")
                nc.vector.tensor_copy(out=rb[:, :], in_=pb[0:64, :])
                ent[2] = rb

            def norm_mul(p, j):
                h = pairs[p][j]
                ent = aph_of[(p, j)]
                t, half = h // 2, (h % 2) * 64
                if half == 0:
                    nc.vector.tensor_mul(out=attnT[t][0:64, :],
                                         in0=ent[0][0:HD, :],
                                         in1=ent[2][:, :])
                else:
                    ah = nrm.tile([64, TOK], BF, tag="ah", name=f"ah{h}")
                    nc.vector.tensor_mul(out=ah[:, :], in0=ent[0][0:HD, :],
                                         in1=ent[2][:, :])
                    nc.sync.dma_start(out=attnT[t][64:128, :], in_=ah[:, :])
                del aph_of[(p, j)]

            def norm_stage2(p, ci):
                if ci == 4:
                    norm_recip(p, 0)
                elif ci == 6:
                    norm_recip(p, 1)
                elif ci == 8:
                    norm_cast(p, 0)
                    norm_cast(p, 1)
                elif ci == 10:
                    norm_bcast(p, 0)
                elif ci == 12:
                    norm_bcast(p, 1)
                    norm_mul(p, 0)
                elif ci == 14:
                    norm_mul(p, 1)

            def emit_scores(pi, ci):
                hA, hB = pairs[pi]
                kt = (hA // GRP) // 2
                qiA, _ = q_slot(hA)
                qiB, _ = q_slot(hB)
                psA = sps.tile([128, 2 * TOK], F32, tag="ps",
                               name=f"psA{hA}_{ci}")
                psB = sps.tile([128, 2 * TOK], F32, tag="ps",
                               name=f"psB{hB}_{ci}")
                for dc in range(2):
                    c = ci + dc
                    nb, lc = c // 4, c % 4
                    kts = kTg[nb * 4 + kt]
                    nc.tensor.matmul(
                        out=psA[:, dc * TOK:(dc + 1) * TOK],
                        lhsT=kts[0:64, lc * 128:(lc + 1) * 128],
                        rhs=qTp[qiA][0:64, :], start=True, stop=True)
                ptA = ptp.tile([128, 2 * TOK], BF, tag="pt",
                               name=f"ptA{hA}_{ci}")
                nc.scalar.activation(
                    out=ptA[:, :], in_=psA[:, :],
                    func=mybir.ActivationFunctionType.Exp, scale=EXP_SCALE)
                for dc in range(2):
                    c = ci + dc
                    nb, lc = c // 4, c % 4
                    kts = kTg[nb * 4 + kt]
                    nc.tensor.matmul(
                        out=psB[:, dc * TOK:(dc + 1) * TOK],
                        lhsT=kts[64:128, lc * 128:(lc + 1) * 128],
                        rhs=qTp[qiB][64:128, :], start=True, stop=True)
                ptB = ptp.tile([128, 2 * TOK], BF, tag="pt",
                               name=f"ptB{hB}_{ci}")
                nc.scalar.activation(
                    out=ptB[:, :], in_=psB[:, :],
                    func=mybir.ActivationFunctionType.Exp, scale=EXP_SCALE)
                return ptA, ptB

            def emit_pv(pi, ci, ptA, ptB):
                hA, hB = pairs[pi]
                khA, khB = hA // GRP, hB // GRP
                poA, poB = po_of[pi]
                for dc in range(2):
                    c = ci + dc
                    nc.tensor.matmul(
                        out=poA[:, :],
                        lhsT=vg[c][:, khA * (HD + 1):(khA + 1) * (HD + 1)],
                        rhs=ptA[:, dc * TOK:(dc + 1) * TOK],
                        start=(c == 0), stop=(c == NKC - 1))
                    nc.tensor.matmul(
                        out=poB[:, :],
                        lhsT=vg[c][:, khB * (HD + 1):(khB + 1) * (HD + 1)],
                        rhs=ptB[:, dc * TOK:(dc + 1) * TOK],
                        start=(c == 0), stop=(c == NKC - 1))

            sched = [(pi, 2 * c2) for pi in range(len(pairs))
                     for c2 in range(NKC // 2)]
            po_of = {}
            inflight = deque()

            for s, (pi, ci) in enumerate(sched):
                hA, hB = pairs[pi]
                if ci == 0:
                    poA = ops.tile([HD + 1, TOK], F32, tag="po",
                                   name=f"poA{hA}")
                    poB = ops.tile([HD + 1, TOK], F32, tag="po",
                                   name=f"poB{hB}")
                    po_of[pi] = (poA, poB)
                # PV of step s-LAG first: it never waits, so it fills the
                # window where scores-A(s) stalls on exp(s-1) freeing the
                # score-PSUM ring (the PE queue is in-order).
                if len(inflight) >= LAG:
                    emit_pv(*inflight.popleft())
                ptA, ptB = emit_scores(pi, ci)
                inflight.append((pi, ci, ptA, ptB))
                if pi > 0:
                    if ci in (2, 4):
                        norm_copy(pi - 1, ci)
                    if ci >= 4:
                        norm_stage2(pi - 1, ci)
                # purely deadline-driven drip pops (deadlines carry the
                # uniform pacing from seal_units)
                while drip and drip[0][0] <= s + 2:
                    _, cost, th = drip.popleft()
                    th()
                # release xin at step 118 (all x/wk/wv consumers done by
                # ~110) and stream the Wo prefetch into the freed SBUF.
                if s == 118:
                    xin_cm.__exit__(None, None, None)
                    wo2_box.append(es.enter_context(
                        tc.tile_pool(name="wo2", bufs=1)))
                    for i, (cst, th) in enumerate(gen_wo_prefetch()):
                        drip.append((119 + i, cst, th))

            while inflight:
                emit_pv(*inflight.popleft())
            while drip:
                drip.popleft()[2]()

        # =============== phase E: output projection + bias ===========
        # sps/dps closed; ops stays open so pair 15's norm drain (which
        # reads po(15)) can overlap E's first 14 kc-groups — attnT[13]
        # and attnT[15] are the only pair-15-gated contraction chunks,
        # so they accumulate last.
        with tc.tile_pool(name="yps", bufs=6, space="PSUM") as yps, \
             tc.tile_pool(name="ystg", bufs=4) as ystg:
            bc_box[0] = (yps, "py")
            kc_order = list(range(13)) + [14, 13, 15]
            for nt in range(4):        # 4 output column blocks of 512
                wo_last = wo_pre[(nt, "last")]
                pys = [yps.tile([128, 512], F32, tag="py",
                                name=f"py{nt}_{i}") for i in range(4)]
                for idx, kc in enumerate(kc_order):
                    wo_t = wo_pre[(nt, kc)]
                    for mt in range(4):
                        nc.tensor.matmul(
                            out=pys[mt][:, :],
                            lhsT=attnT[kc][:, mt * 128:(mt + 1) * 128],
                            rhs=wo_t[:, :],
                            start=(idx == 0), stop=False)
                    if nt == 0 and idx == 13:
                        # pair-15 norm drain: DVE chain runs while the
                        # PE streams the kc-groups emitted above
                        for ci in (2, 4):
                            norm_copy(15, ci)
                        for ci in range(4, 16, 2):
                            norm_stage2(15, ci)
                for mt in range(4):    # bias via ones row, K=1 matmul
                    nc.tensor.matmul(
                        out=pys[mt][:, :], lhsT=ones128[:, :],
                        rhs=wo_last[:, :], start=False, stop=True)
                    ys = ystg.tile([128, 512], F32, tag="ys",
                                   name=f"ys{nt}_{mt}")
                    nc.vector.tensor_copy(out=ys[:, :], in_=pys[mt][:, :])
                    nc.sync.dma_start(
                        out=out[mt * 128:(mt + 1) * 128,
                                nt * 512:(nt + 1) * 512],
                        in_=ys[:, :])
        ops_cm.__exit__(None, None, None)

    nc.finalize()
    return nc


@functools.lru_cache(maxsize=1)
def _graph():
    return build_graph()


def make_in_maps(x, Wq, Wk, Wv, Wo, bo):
    bf16 = ml_dtypes.bfloat16
    x = np.asarray(x, np.float32)
    wqT = np.asarray(Wq, np.float32).T                    # [HID, HID]
    # pre-tiled so one [128, KC*128] DMA loads a whole Q chunk's weights
    wqTk = np.ascontiguousarray(
        wqT.reshape(KC, 128, HID).transpose(1, 0, 2)).astype(bf16)
    wkvT = np.ascontiguousarray(np.concatenate(
        [np.asarray(Wk, np.float32).T, np.asarray(Wv, np.float32).T],
        axis=1)).astype(bf16)                             # [HID, 1024]
    woT = np.concatenate(
        [np.asarray(Wo, np.float32).T,
         np.asarray(bo, np.float32)[None, :]], axis=0).astype(bf16)
    woT = np.ascontiguousarray(woT)
    in_maps = []
    for c in range(8):
        b, r = c // TP, c % TP
        # token permutation: own query block first, rest after (attention
        # is permutation-invariant over keys)
        perm = np.r_[r * TOK:(r + 1) * TOK, 0:r * TOK, (r + 1) * TOK:S]
        xT_c = np.ascontiguousarray(x[b].T[:, perm]).astype(bf16)
        in_maps.append(
            {"xT": xT_c, "wkvT": wkvT, "wqTk": wqTk, "woT": woT})
    return in_maps


def kernel(x, Wq, Wk, Wv, Wo, bo):
    nc = _graph()
    in_maps = make_in_maps(x, Wq, Wk, Wv, Wo, bo)
    res = run_bass_kernel_spmd(nc, in_maps, core_ids=list(range(8)))
    out = np.empty((B, S, HID), np.float32)
    for c in range(8):
        b, r = c // TP, c % TP
        out[b, r * TOK:(r + 1) * TOK, :] = np.asarray(
            res.results[c]["out"], np.float32)
    return out


# revision 33
# speedup vs baseline: 1.1927x; 1.1402x over previous
"""GQA attention (B=2, S=2048, HID=2048, 32 q heads / 8 kv heads, fp32 I/O)
on 8 TRN2 NeuronCores.

Sharding: sequence-parallel with fully local K/V. Core c owns 512 query
tokens of batch c//4 (cores 0-3 = batch 0, cores 4-7 = batch 1), but
computes K^T and V for ALL 2048 tokens of its batch locally — that
(+~40% KV projection FLOPs) is much cheaper than an intra-chip
AllGather, which measures 100-170us and blockades the DMA engines while
it runs. Attention is permutation-invariant over keys, so each core
orders tokens own-block-first (host-side permutation) and the device
program stays rank-independent.

V carries a fused ones-column per kv head so the PV matmul also
produces the softmax row-sums; the output-projection bias is fused as
an extra contraction row. All matmuls run in bf16 with fp32 PSUM
accumulation. Score matmuls (K=64) pair head A (partitions 0-63) and
head B (64-127) on disjoint PE row-groups so the hardware co-executes
them (auto tile_position from base partitions).

v2 schedule: the serial K/V-projection prologue is collapsed to one
8-bank PSUM wave {K mt0 x4, Q0, Q2, V(c0,kv0-3), V(c1,kv0-3)} streamed
chunk-outer against the x DMA arrival, so the first exp fires at
~45us (was ~122us). Everything else — V in 30 finer (chunk, kv-half)
units, K mt1-mt3, Q chunks, Wo prefetch — drips into the ACT-bound
pair loop through a 2-bank ping-pong PSUM ring with deadline-forced,
cost-budgeted pops. The xin pool (x remainder + Wk/Wv) releases at
step 96 to make room for prefetching all 4 Wo column blocks, so the
output projection runs as a pure-PE tail with all 8 PSUM banks.
"""

import functools
from collections import deque
from contextlib import ExitStack

import numpy as np
import ml_dtypes

import concourse.bass as bass
import concourse.mybir as mybir
import concourse.tile as tile
from concourse import bacc
from concourse.bass_utils import run_bass_kernel_spmd

BF = mybir.dt.bfloat16
F32 = mybir.dt.float32

B, S, HID = 2, 2048, 2048
NH, NKV, HD = 32, 8, 64          # q heads, kv heads, head dim
GRP = NH // NKV                  # 4 q heads per kv head
TP = 4                           # cores per batch group
TOK = S // TP                    # 512 local query tokens per core
KC = HID // 128                  # 16 contraction chunks of 128
NKC = S // 128                   # 16 key chunks of 128 (full seq)
VW = NKV * (HD + 1)              # 520: V width incl. ones columns
EXP_SCALE = float(HD) ** -0.5    # 1/8 softmax scale, fused into Exp
LAG = 2                          # steps between scores+exp and its PV


def q_slot(h):
    """qTp tile index and partition base for head h.

    Head h lives at partition base ((h//4)%2)*64 — the same base its kv
    head kh=h//4 occupies inside the kTg tiles, so the scores matmul's
    lhsT and rhs stay partition-aligned (and heads A/B co-execute on
    disjoint PE row groups).
    """
    return ((h // 4) // 2) * 4 + (h % 4), ((h // 4) % 2) * 64


def build_graph():
    nc = bacc.Bacc(None, target_bir_lowering=False, debug=False, num_devices=8)

    # DMA issue slots on the sync queue cost ~650ns EACH regardless of
    # size, so inputs are host-packed for one-issue-per-tile transfers:
    # wkvT = Wk^T|Wv^T fused, wqTk = Wq^T pre-tiled so a whole Q-chunk's
    # 16 weight tiles land in one [128, 16*128] DMA.
    xT = nc.declare_dram_parameter("xT", [HID, S], BF, isOutput=False)
    wkvT = nc.declare_dram_parameter("wkvT", [HID, 2 * NKV * HD], BF,
                                     isOutput=False)
    wqTk = nc.declare_dram_parameter("wqTk", [128, KC, HID], BF,
                                     isOutput=False)
    woT = nc.declare_dram_parameter("woT", [HID + 1, HID], BF, isOutput=False)
    out = nc.declare_dram_parameter("out", [TOK, HID], F32, isOutput=True)

    with tile.TileContext(nc) as tc, ExitStack() as es:
        pers = es.enter_context(tc.tile_pool(name="pers", bufs=1))

        def T(shape, dtype, *, name):
            return pers.tile(shape, dtype, name=name, tag=name)

        # long-lived SBUF pools first; xin LAST so it can release at
        # step 96 while still top-of-stack among SBUF pools.
        wqp = es.enter_context(tc.tile_pool(name="wqp", bufs=4))
        stgB = es.enter_context(tc.tile_pool(name="stgB", bufs=2))
        nrm = es.enter_context(tc.tile_pool(name="nrm", bufs=2))
        ptp = es.enter_context(tc.tile_pool(name="ptp", bufs=6))

        xin_cm = tc.tile_pool(name="xin", bufs=1)
        xin = xin_cm.__enter__()
        xf = [xin.tile([128, S], BF, tag=f"xf{k}", name=f"xf{k}")
              for k in range(KC)]
        wkv = [xin.tile([128, 2 * NKV * HD], BF, tag=f"wkv{k}", name=f"wkv{k}")
               for k in range(KC)]

        def wk_col(k, lo, n):
            return wkv[k][:, lo:lo + n]

        def wv_col(k, lo, n):
            return wkv[k][:, 512 + lo:512 + lo + n]

        # whole-unit weight tiles for wave-0's Q0/Q2
        wq_w0 = {m: wqp.tile([128, KC * 128], BF, tag="wq", name=f"wqw{m}")
                 for m in (0, 2)}

        # ---- DMA issue order = priority. Two issues per x chunk; wq
        # whole-unit tiles first so wave 0's Q matmuls never stall the
        # in-order PE queue.
        for m in (0, 2):
            nc.sync.dma_start(out=wq_w0[m][:, :],
                              in_=wqTk[:, :, m * 128:(m + 1) * 128])
        for k in range(KC):
            nc.sync.dma_start(out=wkv[k][:, :],
                              in_=wkvT[k * 128:(k + 1) * 128, :])
            nc.sync.dma_start(out=xf[k][:, :], in_=xT[k * 128:(k + 1) * 128, :])

        def xcols(k, lo, n):
            # columns lo..lo+n of the permuted x^T chunk k
            return xf[k][:, lo:lo + n]

        # row HD (partition 64) is the K=1 lhsT for the row-sum broadcast
        ones64 = T([HD + 1, 64], BF, name="ones64")
        nc.vector.memset(ones64[:, :], 1.0)
        ones128 = T([1, 128], BF, name="ones128")
        nc.vector.memset(ones128[:, :], 1.0)

        # kTg[nb*4+mt]: [128, 512] = K^T rows mt*128.. for key block nb
        # (kv heads 2mt at partitions 0-63, 2mt+1 at 64-127).
        # vg[c]: [128, 520] V_aug rows for key chunk c, ones at col
        # kh*65+64 of each kv head kh.
        kTg = [T([128, TOK], BF, name=f"kTg{i}") for i in range(16)]
        vg = [T([128, VW], BF, name=f"vg{c}") for c in range(NKC)]
        qTp = [T([128, TOK], BF, name=f"qTp{i}") for i in range(NH // 2)]
        attnT = [T([128, TOK], BF, name=f"attnT{t}") for t in range(NH // 2)]

        def q_fin(ps, m):
            st = stgB.tile([128, TOK], BF, tag="stg", name=f"stq{m}")
            nc.vector.tensor_copy(out=st[:, :], in_=ps[:, :])
            for j in range(2):
                h = 2 * m + j
                i, roff = q_slot(h)
                nc.sync.dma_start(out=qTp[i][roff:roff + 64, :],
                                  in_=st[j * 64:(j + 1) * 64, :])

        def v_fin(ps, c, h):
            if h == 0:
                nc.vector.memset(vg[c][:, :], 1.0)
            for kh in range(4 * h, 4 * h + 4):
                nc.vector.tensor_copy(
                    out=vg[c][:, kh * (HD + 1):kh * (HD + 1) + HD],
                    in_=ps[:, (kh - 4 * h) * 64:(kh - 4 * h + 1) * 64])

        # =============== wave 0: the minimal exp-gating work ============
        # 8 PSUM accumulation groups, contraction-chunk OUTER so the PE
        # streams 8 matmuls per arriving x chunk. Completes ~1.7us after
        # the last x chunk lands; first exp fires ~2us later.
        w0 = ([("k", 0, nb) for nb in range(TP)]
              + [("q", 0, None), ("q", 2, None)]
              + [("v", 0, 0), ("v", 1, 0)])
        with tc.tile_pool(name="accA", bufs=8, space="PSUM") as accA:
            pss = [accA.tile([128, TOK], F32, tag="acc", name=f"psA{i}")
                   for i in range(8)]
            for k in range(KC):
                for ps, (kind, a, b) in zip(pss, w0):
                    if kind == "k":
                        nc.tensor.matmul(
                            out=ps[:, :],
                            lhsT=wk_col(k, 0, 128),
                            rhs=xcols(k, b * TOK, TOK),
                            start=(k == 0), stop=(k == KC - 1))
                    elif kind == "q":
                        nc.tensor.matmul(
                            out=ps[:, :],
                            lhsT=wq_w0[a][:, k * 128:(k + 1) * 128],
                            rhs=xcols(k, 0, TOK),
                            start=(k == 0), stop=(k == KC - 1))
                    else:
                        nc.tensor.matmul(
                            out=ps[:, 0:256],
                            lhsT=xcols(k, a * 128, 128),
                            rhs=wv_col(k, 0, 256),
                            start=(k == 0), stop=(k == KC - 1))
            # evac in exp-gating order: kTg nb0, Q0, Q2, then the rest
            nc.vector.tensor_copy(out=kTg[0][:, :], in_=pss[0][:, :])
            q_fin(pss[4], 0)
            q_fin(pss[5], 2)
            for nb in range(1, TP):
                nc.vector.tensor_copy(out=kTg[nb * 4][:, :],
                                      in_=pss[nb][:, :])
            v_fin(pss[6][:, 0:256], 0, 0)
            v_fin(pss[7][:, 0:256], 1, 0)

        # =============== drip units (fed into the pair loop) ============
        # Each unit: (deadline_step, [(cost_ns, thunk), ...]).
        MM_NS = 216    # 512-col bf16 matmul streaming time
        VMM_NS = 112   # 256-col

        drip = deque()  # (deadline, cost_ns, thunk)
        _units = []     # (deadline, seq, [(cost, thunk), ...])

        def push_unit(deadline, cts):
            _units.append((deadline, len(_units), cts))

        def seal_units():
            # stable-sort by real deadline; pacing happens at pop time
            # via an adaptive quota (see the pair loop)
            total = 0
            for d, _, cts in sorted(_units, key=lambda u: (u[0], u[1])):
                for c, t in cts:
                    drip.append((d, c, t))
                    total += c
            return total

        # unit PSUM tiles MUST allocate lazily at pop time: the dps ring
        # orders its WAR handoffs by .tile() call order, which has to
        # match emission order (norm_bcast's pb tiles share the ring).
        def gen_q_chunk(m):
            """one whole-unit weight DMA, 16 matmuls, finisher."""
            box = {}
            ws = wqp.tile([128, KC * 128], BF, tag="wq", name=f"wqu{m}")
            def dm(m=m):
                nc.sync.dma_start(out=ws[:, :],
                                  in_=wqTk[:, :, m * 128:(m + 1) * 128])
            thunks = [(0, dm)]
            for k in range(KC):
                def mm(k=k, m=m):
                    if k == 0:
                        box["ps"] = dps.tile([128, TOK], F32, tag="dp",
                                             name=f"psq{m}")
                    nc.tensor.matmul(
                        out=box["ps"][:, :],
                        lhsT=ws[:, k * 128:(k + 1) * 128],
                        rhs=xcols(k, 0, TOK),
                        start=(k == 0), stop=(k == KC - 1))
                thunks.append((MM_NS, mm))
            thunks.append((0, lambda m=m: q_fin(box["ps"], m)))
            return thunks

        def gen_k_unit(mt, nb):
            box = {}
            thunks = []
            for k in range(KC):
                def mm(k=k, mt=mt, nb=nb):
                    if k == 0:
                        box["ps"] = dps.tile([128, TOK], F32, tag="dp",
                                             name=f"psk{nb}_{mt}")
                    nc.tensor.matmul(
                        out=box["ps"][:, :],
                        lhsT=wk_col(k, mt * 128, 128),
                        rhs=xcols(k, nb * TOK, TOK),
                        start=(k == 0), stop=(k == KC - 1))
                thunks.append((MM_NS, mm))
            def fin(mt=mt, nb=nb):
                nc.vector.tensor_copy(out=kTg[nb * 4 + mt][:, :],
                                      in_=box["ps"][:, :])
            thunks.append((0, fin))
            return thunks

        def gen_v_unit(c, h):
            box = {}
            thunks = []
            for k in range(KC):
                def mm(k=k, c=c, h=h):
                    if k == 0:
                        box["ps"] = dps.tile([128, TOK], F32, tag="dp",
                                             name=f"psv{c}_{h}")
                    nc.tensor.matmul(
                        out=box["ps"][:, 0:256],
                        lhsT=xcols(k, c * 128, 128),
                        rhs=wv_col(k, h * 256, 256),
                        start=(k == 0), stop=(k == KC - 1))
                thunks.append((VMM_NS, mm))
            thunks.append(
                (0, lambda c=c, h=h: v_fin(box["ps"][:, 0:256], c, h)))
            return thunks

        # Wo prefetch (DMA-only): queued when xin releases at step 118.
        # One [128, 2048] row-block DMA per kc (all 4 nt at once).
        wo_pre = {}
        wo2_box = []

        def gen_wo_prefetch():
            thunks = []
            def last():
                wl = wo2_box[0].tile([1, HID], BF, tag="wolast",
                                     bufs=1, name="wl")
                nc.sync.dma_start(out=wl[:, :], in_=woT[HID:HID + 1, :])
                for nt in range(4):
                    wo_pre[(nt, "last")] = wl[:, nt * 512:(nt + 1) * 512]
            thunks.append((0, last))
            for kc in range(KC):
                def f(kc=kc):
                    w = wo2_box[0].tile([128, HID], BF, tag="wo", bufs=16,
                                        name=f"wo{kc}")
                    nc.sync.dma_start(
                        out=w[:, :], in_=woT[kc * 128:(kc + 1) * 128, :])
                    for nt in range(4):
                        wo_pre[(nt, kc)] = w[:, nt * 512:(nt + 1) * 512]
                thunks.append((0, f))
            return thunks

        # =============== the pair loop ==================================
        pairs = []
        for g in range(0, NKV, 2):
            for j in range(GRP):
                pairs.append((g * GRP + j, (g + 1) * GRP + j))

        bc_box = [None]  # (pool, tag) for the row-sum broadcast PSUM
        ops_cm = tc.tile_pool(name="ops", bufs=2, space="PSUM")
        ops = ops_cm.__enter__()
        with tc.tile_pool(name="sps", bufs=2, space="PSUM") as sps, \
             tc.tile_pool(name="dps", bufs=2, space="PSUM") as dps:
            bc_box[0] = (dps, "dp")

            # drip queue in deadline order
            def push_q(d, m):
                ths = gen_q_chunk(m)
                # weight DMA leads its matmuls by ~4 steps so the 512KB
                # transfer never head-of-line blocks the PE queue
                push_unit(max(0, d - 4), [ths[0]])
                push_unit(d, ths[1:])

            for c in range(2, NKC):                      # V kv0-3 rest
                push_unit(c // 2 + 1, gen_v_unit(c, 0))
            push_q(13, 1)
            push_q(13, 3)
            for nb in range(TP):                         # K mt1
                push_unit(28 + 2 * nb, gen_k_unit(1, nb))
            push_q(29, 4)
            push_q(29, 6)
            push_q(44, 5)
            push_q(44, 7)
            for c in range(NKC):                         # V kv4-7
                push_unit(61 + c // 2, gen_v_unit(c, 1))
            for nb in range(TP):                         # K mt2
                push_unit(61 + 2 * nb, gen_k_unit(2, nb))
            push_q(61, 8)
            push_q(61, 10)
            push_q(76, 9)
            push_q(76, 11)
            for nb in range(TP):                         # K mt3
                push_unit(92 + 2 * nb, gen_k_unit(3, nb))
            push_q(93, 12)
            push_q(93, 14)
            push_q(105, 13)
            push_q(105, 15)
            rem_cost = seal_units()

            # Normalization for pair p staged across pair p+1's steps
            # (PSUM->SBUF copy, reciprocal, cast, PE ones-broadcast via
            # the dps ring, multiply) so the 3.3us DVE reciprocal never
            # blocks a PE-side consumer.
            aph_of = {}

            def norm_copy(p, ci):
                j = 0 if ci == 2 else 1
                h = pairs[p][j]
                po = po_of[p][j]
                aph = nrm.tile([HD + 1, TOK], F32, tag="aph", bufs=2,
                               name=f"aph{h}")
                nc.vector.tensor_copy(out=aph[:, :], in_=po[:, :])
                aph_of[(p, j)] = [aph, None, None]
                if ci == 4:
                    del po_of[p]

            def norm_recip(p, j):
                h = pairs[p][j]
                ent = aph_of[(p, j)]
                rcp = nrm.tile([HD + 1, TOK], F32, tag="rcp", name=f"rc{h}")
                nc.vector.reciprocal(out=rcp[HD:HD + 1, :],
                                     in_=ent[0][HD:HD + 1, :])
                ent[1] = rcp

            def norm_cast(p, j):
                h = pairs[p][j]
                ent = aph_of[(p, j)]
                rcpb = nrm.tile([HD + 1, TOK], BF, tag="rcpb", name=f"rb{h}")
                nc.vector.tensor_copy(out=rcpb[HD:HD + 1, :],
                                      in_=ent[1][HD:HD + 1, :])
                ent[1] = rcpb

            def norm_bcast(p, j):
                h = pairs[p][j]
                ent = aph_of[(p, j)]
                pool, tag = bc_box[0]
                pb = pool.tile([128, TOK], F32, tag=tag, name=f"pb{h}")
                nc.tensor.matmul(out=pb[0:64, :], lhsT=ones64[HD:HD + 1, :],
                                 rhs=ent[1][HD:HD + 1, :],
                                 start=True, stop=True)
                rb = nrm.tile([64, TOK], BF, tag="rbb", name=f"rbb{h}")
                nc.vector.tensor_copy(out=rb[:, :], in_=pb[0:64, :])
                ent[2] = rb

            def norm_mul(p, j):
                h = pairs[p][j]
                ent = aph_of[(p, j)]
                t, half = h // 2, (h % 2) * 64
                if half == 0:
                    nc.vector.tensor_mul(out=attnT[t][0:64, :],
                                         in0=ent[0][0:HD, :],
                                         in1=ent[2][:, :])
                else:
                    ah = nrm.tile([64, TOK], BF, tag="ah", name=f"ah{h}")
                    nc.vector.tensor_mul(out=ah[:, :], in0=ent[0][0:HD, :],
                                         in1=ent[2][:, :])
                    nc.sync.dma_start(out=attnT[t][64:128, :], in_=ah[:, :])
                del aph_of[(p, j)]

            def norm_stage2(p, ci):
                if ci == 4:
                    norm_recip(p, 0)
                elif ci == 6:
                    norm_recip(p, 1)
                elif ci == 8:
                    norm_cast(p, 0)
                    norm_cast(p, 1)
                elif ci == 10:
                    norm_bcast(p, 0)
                elif ci == 12:
                    norm_bcast(p, 1)
                    norm_mul(p, 0)
                elif ci == 14:
                    norm_mul(p, 1)

            def emit_scores(pi, ci):
                hA, hB = pairs[pi]
                kt = (hA // GRP) // 2
                qiA, _ = q_slot(hA)
                qiB, _ = q_slot(hB)
                psA = sps.tile([128, 2 * TOK], F32, tag="ps",
                               name=f"psA{hA}_{ci}")
                psB = sps.tile([128, 2 * TOK], F32, tag="ps",
                               name=f"psB{hB}_{ci}")
                for dc in range(2):
                    c = ci + dc
                    nb, lc = c // 4, c % 4
                    kts = kTg[nb * 4 + kt]
                    nc.tensor.matmul(
                        out=psA[:, dc * TOK:(dc + 1) * TOK],
                        lhsT=kts[0:64, lc * 128:(lc + 1) * 128],
                        rhs=qTp[qiA][0:64, :], start=True, stop=True)
                ptA = ptp.tile([128, 2 * TOK], BF, tag="pt",
                               name=f"ptA{hA}_{ci}")
                nc.scalar.activation(
                    out=ptA[:, :], in_=psA[:, :],
                    func=mybir.ActivationFunctionType.Exp, scale=EXP_SCALE)
                for dc in range(2):
                    c = ci + dc
                    nb, lc = c // 4, c % 4
                    kts = kTg[nb * 4 + kt]
                    nc.tensor.matmul(
                        out=psB[:, dc * TOK:(dc + 1) * TOK],
                        lhsT=kts[64:128, lc * 128:(lc + 1) * 128],
                        rhs=qTp[qiB][64:128, :], start=True, stop=True)
                ptB = ptp.tile([128, 2 * TOK], BF, tag="pt",
                               name=f"ptB{hB}_{ci}")
                nc.scalar.activation(
                    out=ptB[:, :], in_=psB[:, :],
                    func=mybir.ActivationFunctionType.Exp, scale=EXP_SCALE)
                return ptA, ptB

            def emit_pv(pi, ci, ptA, ptB):
                hA, hB = pairs[pi]
                khA, khB = hA // GRP, hB // GRP
                poA, poB = po_of[pi]
                for dc in range(2):
                    c = ci + dc
                    nc.tensor.matmul(
                        out=poA[:, :],
                        lhsT=vg[c][:, khA * (HD + 1):(khA + 1) * (HD + 1)],
                        rhs=ptA[:, dc * TOK:(dc + 1) * TOK],
                        start=(c == 0), stop=(c == NKC - 1))
                    nc.tensor.matmul(
                        out=poB[:, :],
                        lhsT=vg[c][:, khB * (HD + 1):(khB + 1) * (HD + 1)],
                        rhs=ptB[:, dc * TOK:(dc + 1) * TOK],
                        start=(c == 0), stop=(c == NKC - 1))

            sched = [(pi, 2 * c2) for pi in range(len(pairs))
                     for c2 in range(NKC // 2)]
            po_of = {}
            inflight = deque()

            for s, (pi, ci) in enumerate(sched):
                hA, hB = pairs[pi]
                if ci == 0:
                    poA = ops.tile([HD + 1, TOK], F32, tag="po",
                                   name=f"poA{hA}")
                    poB = ops.tile([HD + 1, TOK], F32, tag="po",
                                   name=f"poB{hB}")
                    po_of[pi] = (poA, poB)
                # PV of step s-LAG first: it never waits, so it fills the
                # window where scores-A(s) stalls on exp(s-1) freeing the
                # score-PSUM ring (the PE queue is in-order).
                if len(inflight) >= LAG:
                    emit_pv(*inflight.popleft())
                ptA, ptB = emit_scores(pi, ci)
                inflight.append((pi, ci, ptA, ptB))
                if pi > 0:
                    if ci in (2, 4):
                        norm_copy(pi - 1, ci)
                    if ci >= 4:
                        norm_stage2(pi - 1, ci)
                # drip pops: real deadlines force correctness-critical
                # work; the adaptive quota spreads everything else so
                # the queue neither bursts (burying the next scores in
                # the in-order PE queue) nor runs dry (PE p-state drop).
                due = 0
                for dd, cc, _ in drip:
                    if dd > s + 25:
                        break
                    due += cc
                quota = max(rem_cost / max(1.0, 126.0 - s), due / 25.0)
                spent = 0
                while drip and (drip[0][0] <= s + 2 or spent < quota):
                    _, cost, th = drip.popleft()
                    th()
                    spent += cost
                    rem_cost -= cost
                # release xin at step 118 (all x/wk/wv consumers done by
                # ~110) and stream the Wo prefetch into the freed SBUF.
                if s == 118:
                    xin_cm.__exit__(None, None, None)
                    wo2_box.append(es.enter_context(
                        tc.tile_pool(name="wo2", bufs=1)))
                    for i, (cst, th) in enumerate(gen_wo_prefetch()):
                        drip.append((119 + i, cst, th))

            while inflight:
                emit_pv(*inflight.popleft())
            while drip:
                drip.popleft()[2]()

        # =============== phase E: output projection + bias ===========
        # sps/dps closed; ops stays open so pair 15's norm drain (which
        # reads po(15)) can overlap E's first 14 kc-groups — attnT[13]
        # and attnT[15] are the only pair-15-gated contraction chunks,
        # so they accumulate last.
        with tc.tile_pool(name="yps", bufs=6, space="PSUM") as yps, \
             tc.tile_pool(name="ystg", bufs=4) as ystg:
            bc_box[0] = (yps, "py")
            kc_order = list(range(13)) + [14, 13, 15]
            for nt in range(4):        # 4 output column blocks of 512
                wo_last = wo_pre[(nt, "last")]
                pys = [yps.tile([128, 512], F32, tag="py",
                                name=f"py{nt}_{i}") for i in range(4)]
                for idx, kc in enumerate(kc_order):
                    wo_t = wo_pre[(nt, kc)]
                    for mt in range(4):
                        nc.tensor.matmul(
                            out=pys[mt][:, :],
                            lhsT=attnT[kc][:, mt * 128:(mt + 1) * 128],
                            rhs=wo_t[:, :],
                            start=(idx == 0), stop=False)
                    if nt == 0 and idx == 13:
                        # pair-15 norm drain: DVE chain runs while the
                        # PE streams the kc-groups emitted above
                        for ci in (2, 4):
                            norm_copy(15, ci)
                        for ci in range(4, 16, 2):
                            norm_stage2(15, ci)
                for mt in range(4):    # bias via ones row, K=1 matmul
                    nc.tensor.matmul(
                        out=pys[mt][:, :], lhsT=ones128[:, :],
                        rhs=wo_last[:, :], start=False, stop=True)
                    ys = ystg.tile([128, 512], F32, tag="ys",
                                   name=f"ys{nt}_{mt}")
                    nc.vector.tensor_copy(out=ys[:, :], in_=pys[mt][:, :])
                    nc.sync.dma_start(
                        out=out[mt * 128:(mt + 1) * 128,
                                nt * 512:(nt + 1) * 512],
                        in_=ys[:, :])
        ops_cm.__exit__(None, None, None)

    nc.finalize()
    return nc


@functools.lru_cache(maxsize=1)
def _graph():
    return build_graph()


def make_in_maps(x, Wq, Wk, Wv, Wo, bo):
    bf16 = ml_dtypes.bfloat16
    x = np.asarray(x, np.float32)
    wqT = np.asarray(Wq, np.float32).T                    # [HID, HID]
    # pre-tiled so one [128, KC*128] DMA loads a whole Q chunk's weights
    wqTk = np.ascontiguousarray(
        wqT.reshape(KC, 128, HID).transpose(1, 0, 2)).astype(bf16)
    wkvT = np.ascontiguousarray(np.concatenate(
        [np.asarray(Wk, np.float32).T, np.asarray(Wv, np.float32).T],
        axis=1)).astype(bf16)                             # [HID, 1024]
    woT = np.concatenate(
        [np.asarray(Wo, np.float32).T,
         np.asarray(bo, np.float32)[None, :]], axis=0).astype(bf16)
    woT = np.ascontiguousarray(woT)
    in_maps = []
    for c in range(8):
        b, r = c // TP, c % TP
        # token permutation: own query block first, rest after (attention
        # is permutation-invariant over keys)
        perm = np.r_[r * TOK:(r + 1) * TOK, 0:r * TOK, (r + 1) * TOK:S]
        xT_c = np.ascontiguousarray(x[b].T[:, perm]).astype(bf16)
        in_maps.append(
            {"xT": xT_c, "wkvT": wkvT, "wqTk": wqTk, "woT": woT})
    return in_maps


def kernel(x, Wq, Wk, Wv, Wo, bo):
    nc = _graph()
    in_maps = make_in_maps(x, Wq, Wk, Wv, Wo, bo)
    res = run_bass_kernel_spmd(nc, in_maps, core_ids=list(range(8)))
    out = np.empty((B, S, HID), np.float32)
    for c in range(8):
        b, r = c // TP, c % TP
        out[b, r * TOK:(r + 1) * TOK, :] = np.asarray(
            res.results[c]["out"], np.float32)
    return out


# revision 36
# speedup vs baseline: 1.3212x; 1.1078x over previous
"""GQA attention (B=2, S=2048, HID=2048, 32 q heads / 8 kv heads, fp32 I/O)
on 8 TRN2 NeuronCores.

Sharding: sequence-parallel with fully local K/V. Core c owns 512 query
tokens of batch c//4 (cores 0-3 = batch 0, cores 4-7 = batch 1), but
computes K^T and V for ALL 2048 tokens of its batch locally — that
(+~40% KV projection FLOPs) is much cheaper than an intra-chip
AllGather, which measures 100-170us and blockades the DMA engines while
it runs. Attention is permutation-invariant over keys, so each core
orders tokens own-block-first (host-side permutation) and the device
program stays rank-independent.

V carries a fused ones-column per kv head so the PV matmul also
produces the softmax row-sums; the output-projection bias is fused as
an extra contraction row. All matmuls run in bf16 with fp32 PSUM
accumulation. Score matmuls (K=64) pair head A (partitions 0-63) and
head B (64-127) on disjoint PE row-groups so the hardware co-executes
them (auto tile_position from base partitions).

v2 schedule: the serial K/V-projection prologue is collapsed to one
8-bank PSUM wave {K mt0 x4, Q0, Q2, V(c0,kv0-3), V(c1,kv0-3)} streamed
chunk-outer against the x DMA arrival, so the first exp fires at
~45us (was ~122us). Everything else — V in 30 finer (chunk, kv-half)
units, K mt1-mt3, Q chunks, Wo prefetch — drips into the ACT-bound
pair loop through a 2-bank ping-pong PSUM ring with deadline-forced,
cost-budgeted pops. The xin pool (x remainder + Wk/Wv) releases at
step 96 to make room for prefetching all 4 Wo column blocks, so the
output projection runs as a pure-PE tail with all 8 PSUM banks.
"""

import functools
from collections import deque
from contextlib import ExitStack

import numpy as np
import ml_dtypes

import concourse.bass as bass
import concourse.mybir as mybir
import concourse.tile as tile
from concourse import bacc
from concourse.bass_utils import run_bass_kernel_spmd

BF = mybir.dt.bfloat16
F32 = mybir.dt.float32

B, S, HID = 2, 2048, 2048
NH, NKV, HD = 32, 8, 64          # q heads, kv heads, head dim
GRP = NH // NKV                  # 4 q heads per kv head
TP = 4                           # cores per batch group
TOK = S // TP                    # 512 local query tokens per core
KC = HID // 128                  # 16 contraction chunks of 128
NKC = S // 128                   # 16 key chunks of 128 (full seq)
VW = NKV * (HD + 1)              # 520: V width incl. ones columns
EXP_SCALE = float(HD) ** -0.5    # 1/8 softmax scale, fused into Exp
LAG = 2                          # steps between scores+exp and its PV


def q_slot(h):
    """qTp tile index and partition base for head h.

    Head h lives at partition base ((h//4)%2)*64 — the same base its kv
    head kh=h//4 occupies inside the kTg tiles, so the scores matmul's
    lhsT and rhs stay partition-aligned (and heads A/B co-execute on
    disjoint PE row groups).
    """
    return ((h // 4) // 2) * 4 + (h % 4), ((h // 4) % 2) * 64


def build_graph():
    nc = bacc.Bacc(None, target_bir_lowering=False, debug=False, num_devices=8)

    # DMA issue slots on the sync queue cost ~650ns EACH regardless of
    # size, so inputs are host-packed for one-issue-per-tile transfers:
    # wkvT = Wk^T|Wv^T fused, wqTk = Wq^T pre-tiled so a whole Q-chunk's
    # 16 weight tiles land in one [128, 16*128] DMA.
    xT = nc.declare_dram_parameter("xT", [HID, S], BF, isOutput=False)
    wkvT = nc.declare_dram_parameter("wkvT", [HID, 2 * NKV * HD], BF,
                                     isOutput=False)
    wqTk = nc.declare_dram_parameter("wqTk", [128, KC, HID], BF,
                                     isOutput=False)
    woT = nc.declare_dram_parameter("woT", [HID + 1, HID], BF, isOutput=False)
    out = nc.declare_dram_parameter("out", [TOK, HID], F32, isOutput=True)

    with tile.TileContext(nc) as tc, ExitStack() as es:
        pers = es.enter_context(tc.tile_pool(name="pers", bufs=1))

        def T(shape, dtype, *, name):
            return pers.tile(shape, dtype, name=name, tag=name)

        # long-lived SBUF pools first; xin LAST so it can release at
        # step 96 while still top-of-stack among SBUF pools.
        wqp = es.enter_context(tc.tile_pool(name="wqp", bufs=4))
        stgB = es.enter_context(tc.tile_pool(name="stgB", bufs=2))
        nrm = es.enter_context(tc.tile_pool(name="nrm", bufs=2))
        ptp = es.enter_context(tc.tile_pool(name="ptp", bufs=6))

        xin_cm = tc.tile_pool(name="xin", bufs=1)
        xin = xin_cm.__enter__()
        xf = [xin.tile([128, S], BF, tag=f"xf{k}", name=f"xf{k}")
              for k in range(KC)]
        wkv = [xin.tile([128, 2 * NKV * HD], BF, tag=f"wkv{k}", name=f"wkv{k}")
               for k in range(KC)]

        def wk_col(k, lo, n):
            return wkv[k][:, lo:lo + n]

        def wv_col(k, lo, n):
            return wkv[k][:, 512 + lo:512 + lo + n]

        # whole-unit weight tiles for wave-0's Q0/Q2
        wq_w0 = {m: wqp.tile([128, KC * 128], BF, tag="wq", name=f"wqw{m}")
                 for m in (0, 2)}

        # ---- DMA issue order = priority. Two issues per x chunk; wq
        # whole-unit tiles first so wave 0's Q matmuls never stall the
        # in-order PE queue.
        for m in (0, 2):
            nc.sync.dma_start(out=wq_w0[m][:, :],
                              in_=wqTk[:, :, m * 128:(m + 1) * 128])
        for k in range(KC):
            nc.sync.dma_start(out=wkv[k][:, :],
                              in_=wkvT[k * 128:(k + 1) * 128, :])
            nc.sync.dma_start(out=xf[k][:, :], in_=xT[k * 128:(k + 1) * 128, :])

        def xcols(k, lo, n):
            # columns lo..lo+n of the permuted x^T chunk k
            return xf[k][:, lo:lo + n]

        # row HD (partition 64) is the K=1 lhsT for the row-sum broadcast
        ones64 = T([HD + 1, 64], BF, name="ones64")
        nc.vector.memset(ones64[:, :], 1.0)
        ones128 = T([1, 128], BF, name="ones128")
        nc.vector.memset(ones128[:, :], 1.0)
        ones512 = T([1, TOK], BF, name="ones512")
        nc.vector.memset(ones512[:, :], 1.0)

        # kTg[nb*4+mt]: [128, 512] = K^T rows mt*128.. for key block nb
        # (kv heads 2mt at partitions 0-63, 2mt+1 at 64-127).
        # vg[c]: [128, 520] V_aug rows for key chunk c, ones at col
        # kh*65+64 of each kv head kh.
        kTg = [T([128, TOK], BF, name=f"kTg{i}") for i in range(16)]
        vg = [T([128, VW], BF, name=f"vg{c}") for c in range(NKC)]
        qTp = [T([128, TOK], BF, name=f"qTp{i}") for i in range(NH // 2)]
        attnT = [T([128, TOK], BF, name=f"attnT{t}") for t in range(NH // 2)]

        def q_fin(ps, m):
            st = stgB.tile([128, TOK], BF, tag="stg", name=f"stq{m}")
            nc.vector.tensor_copy(out=st[:, :], in_=ps[:, :])
            for j in range(2):
                h = 2 * m + j
                i, roff = q_slot(h)
                nc.sync.dma_start(out=qTp[i][roff:roff + 64, :],
                                  in_=st[j * 64:(j + 1) * 64, :])

        def v_fin(ps, c, h):
            if h == 0:
                nc.vector.memset(vg[c][:, :], 1.0)
            for kh in range(4 * h, 4 * h + 4):
                nc.vector.tensor_copy(
                    out=vg[c][:, kh * (HD + 1):kh * (HD + 1) + HD],
                    in_=ps[:, (kh - 4 * h) * 64:(kh - 4 * h + 1) * 64])

        # =============== wave 0: the minimal exp-gating work ============
        # 8 PSUM accumulation groups, contraction-chunk OUTER so the PE
        # streams 8 matmuls per arriving x chunk. Completes ~1.7us after
        # the last x chunk lands; first exp fires ~2us later.
        w0 = ([("k", 0, nb) for nb in range(TP)]
              + [("q", 0, None), ("q", 2, None)]
              + [("v", 0, 0), ("v", 1, 0)])
        with tc.tile_pool(name="accA", bufs=8, space="PSUM") as accA:
            pss = [accA.tile([128, TOK], F32, tag="acc", name=f"psA{i}")
                   for i in range(8)]
            for k in range(KC):
                for ps, (kind, a, b) in zip(pss, w0):
                    if kind == "k":
                        nc.tensor.matmul(
                            out=ps[:, :],
                            lhsT=wk_col(k, 0, 128),
                            rhs=xcols(k, b * TOK, TOK),
                            start=(k == 0), stop=(k == KC - 1))
                    elif kind == "q":
                        nc.tensor.matmul(
                            out=ps[:, :],
                            lhsT=wq_w0[a][:, k * 128:(k + 1) * 128],
                            rhs=xcols(k, 0, TOK),
                            start=(k == 0), stop=(k == KC - 1))
                    else:
                        nc.tensor.matmul(
                            out=ps[:, 0:256],
                            lhsT=xcols(k, a * 128, 128),
                            rhs=wv_col(k, 0, 256),
                            start=(k == 0), stop=(k == KC - 1))
            # evac in exp-gating order: kTg nb0, Q0, Q2, then the rest
            nc.vector.tensor_copy(out=kTg[0][:, :], in_=pss[0][:, :])
            q_fin(pss[4], 0)
            q_fin(pss[5], 2)
            for nb in range(1, TP):
                nc.vector.tensor_copy(out=kTg[nb * 4][:, :],
                                      in_=pss[nb][:, :])
            v_fin(pss[6][:, 0:256], 0, 0)
            v_fin(pss[7][:, 0:256], 1, 0)

        # =============== drip units (fed into the pair loop) ============
        # Each unit: (deadline_step, [(cost_ns, thunk), ...]).
        MM_NS = 216    # 512-col bf16 matmul streaming time
        VMM_NS = 112   # 256-col

        drip = deque()  # (deadline, cost_ns, thunk)
        _units = []     # (deadline, seq, [(cost, thunk), ...])

        def push_unit(deadline, cts):
            _units.append((deadline, len(_units), cts))

        def seal_units():
            # stable-sort by real deadline; pacing happens at pop time
            # via an adaptive quota (see the pair loop)
            total = 0
            for d, _, cts in sorted(_units, key=lambda u: (u[0], u[1])):
                for c, t in cts:
                    drip.append((d, c, t))
                    total += c
            return total

        # unit PSUM tiles MUST allocate lazily at pop time: the dps ring
        # orders its WAR handoffs by .tile() call order, which has to
        # match emission order (norm_bcast's pb tiles share the ring).
        def gen_q_chunk(m):
            """one whole-unit weight DMA, 16 matmuls, finisher."""
            box = {}
            ws = wqp.tile([128, KC * 128], BF, tag="wq", name=f"wqu{m}")
            def dm(m=m):
                nc.sync.dma_start(out=ws[:, :],
                                  in_=wqTk[:, :, m * 128:(m + 1) * 128])
            thunks = [(0, dm)]
            for k in range(KC):
                def mm(k=k, m=m):
                    if k == 0:
                        box["ps"] = dps.tile([128, TOK], F32, tag="dp",
                                             name=f"psq{m}")
                    nc.tensor.matmul(
                        out=box["ps"][:, :],
                        lhsT=ws[:, k * 128:(k + 1) * 128],
                        rhs=xcols(k, 0, TOK),
                        start=(k == 0), stop=(k == KC - 1))
                thunks.append((MM_NS, mm))
            thunks.append((0, lambda m=m: q_fin(box["ps"], m)))
            return thunks

        def gen_k_unit(mt, nb):
            box = {}
            thunks = []
            for k in range(KC):
                def mm(k=k, mt=mt, nb=nb):
                    if k == 0:
                        box["ps"] = dps.tile([128, TOK], F32, tag="dp",
                                             name=f"psk{nb}_{mt}")
                    nc.tensor.matmul(
                        out=box["ps"][:, :],
                        lhsT=wk_col(k, mt * 128, 128),
                        rhs=xcols(k, nb * TOK, TOK),
                        start=(k == 0), stop=(k == KC - 1))
                thunks.append((MM_NS, mm))
            def fin(mt=mt, nb=nb):
                nc.vector.tensor_copy(out=kTg[nb * 4 + mt][:, :],
                                      in_=box["ps"][:, :])
            thunks.append((0, fin))
            return thunks

        def gen_v_unit(c, h):
            box = {}
            thunks = []
            for k in range(KC):
                def mm(k=k, c=c, h=h):
                    if k == 0:
                        box["ps"] = dps.tile([128, TOK], F32, tag="dp",
                                             name=f"psv{c}_{h}")
                    nc.tensor.matmul(
                        out=box["ps"][:, 0:256],
                        lhsT=xcols(k, c * 128, 128),
                        rhs=wv_col(k, h * 256, 256),
                        start=(k == 0), stop=(k == KC - 1))
                thunks.append((VMM_NS, mm))
            thunks.append(
                (0, lambda c=c, h=h: v_fin(box["ps"][:, 0:256], c, h)))
            return thunks

        # Wo prefetch (DMA-only): queued when xin releases at step 118.
        # One [128, 2048] row-block DMA per kc (all 4 nt at once).
        wo_pre = {}
        wo2_box = []

        def gen_wo_prefetch():
            thunks = []
            def last():
                wl = wo2_box[0].tile([1, HID], BF, tag="wolast",
                                     bufs=1, name="wl")
                nc.sync.dma_start(out=wl[:, :], in_=woT[HID:HID + 1, :])
                for nt in range(4):
                    wo_pre[(nt, "last")] = wl[:, nt * 512:(nt + 1) * 512]
            thunks.append((0, last))
            for kc in range(KC):
                def f(kc=kc):
                    w = wo2_box[0].tile([128, HID], BF, tag="wo", bufs=16,
                                        name=f"wo{kc}")
                    nc.sync.dma_start(
                        out=w[:, :], in_=woT[kc * 128:(kc + 1) * 128, :])
                    for nt in range(4):
                        wo_pre[(nt, kc)] = w[:, nt * 512:(nt + 1) * 512]
                thunks.append((0, f))
            return thunks

        # =============== the pair loop ==================================
        pairs = []
        for g in range(0, NKV, 2):
            for j in range(GRP):
                pairs.append((g * GRP + j, (g + 1) * GRP + j))

        bc_box = [None]  # (pool, tag) for the row-sum broadcast PSUM
        ops_cm = tc.tile_pool(name="ops", bufs=2, space="PSUM")
        ops = ops_cm.__enter__()
        with tc.tile_pool(name="sps", bufs=2, space="PSUM") as sps, \
             tc.tile_pool(name="dps", bufs=2, space="PSUM") as dps:
            bc_box[0] = (dps, "dp")

            # drip queue in deadline order
            def push_q(d, m):
                ths = gen_q_chunk(m)
                # weight DMA leads its matmuls by ~4 steps so the 512KB
                # transfer never head-of-line blocks the PE queue
                push_unit(max(0, d - 4), [ths[0]])
                push_unit(d, ths[1:])

            for c in range(2, NKC):                      # V kv0-3 rest
                push_unit(c // 2 + 1, gen_v_unit(c, 0))
            push_q(13, 1)
            push_q(13, 3)
            for nb in range(TP):                         # K mt1
                push_unit(28 + 2 * nb, gen_k_unit(1, nb))
            push_q(29, 4)
            push_q(29, 6)
            push_q(44, 5)
            push_q(44, 7)
            for c in range(NKC):                         # V kv4-7
                push_unit(61 + c // 2, gen_v_unit(c, 1))
            for nb in range(TP):                         # K mt2
                push_unit(61 + 2 * nb, gen_k_unit(2, nb))
            push_q(61, 8)
            push_q(61, 10)
            push_q(76, 9)
            push_q(76, 11)
            for nb in range(TP):                         # K mt3
                push_unit(92 + 2 * nb, gen_k_unit(3, nb))
            push_q(93, 12)
            push_q(93, 14)
            push_q(105, 13)
            push_q(105, 15)
            rem_cost = seal_units()

            # Normalization for pair p staged across pair p+1's steps
            # (PSUM->SBUF copy, reciprocal, cast, PE ones-broadcast via
            # the dps ring, multiply) so the 3.3us DVE reciprocal never
            # blocks a PE-side consumer.
            aph_of = {}

            def norm_copy(p, ci):
                j = 0 if ci == 2 else 1
                h = pairs[p][j]
                po = po_of[p][j]
                aph = nrm.tile([HD + 1, TOK], F32, tag="aph", bufs=2,
                               name=f"aph{h}")
                nc.vector.tensor_copy(out=aph[:, :], in_=po[:, :])
                aph_of[(p, j)] = [aph, None, None]
                if ci == 4:
                    del po_of[p]

            def norm_recip(p, j):
                # DVE reciprocal is an iterative divide, and the row sum
                # lives on ONE partition: [1,512] costs 3.3us on a single
                # lane. Spread it to [128,4] via a reshaping SBUF->SBUF
                # DMA so all 128 lanes divide in parallel (~0.15us).
                h = pairs[p][j]
                ent = aph_of[(p, j)]
                z4 = nrm.tile([128, 4], F32, tag="z4", name=f"z4{h}")
                nc.sync.dma_start(out=z4[:, :], in_=ent[0][HD:HD + 1, :])
                r4 = nrm.tile([128, 4], F32, tag="r4", name=f"r4{h}")
                nc.vector.reciprocal(out=r4[:, :], in_=z4[:, :])
                rb4 = nrm.tile([128, 4], BF, tag="rb4", name=f"rb4{h}")
                nc.vector.tensor_copy(out=rb4[:, :], in_=r4[:, :])
                ent[1] = rb4

            def norm_cast(p, j):
                # gather the spread reciprocal back to [1,512] bf16
                h = pairs[p][j]
                ent = aph_of[(p, j)]
                rcpb = nrm.tile([HD + 1, TOK], BF, tag="rcpb", name=f"rb{h}")
                nc.sync.dma_start(out=rcpb[HD:HD + 1, :], in_=ent[1][:, :])
                ent[1] = rcpb

            def norm_bcast(p, j):
                h = pairs[p][j]
                ent = aph_of[(p, j)]
                pool, tag = bc_box[0]
                pb = pool.tile([128, TOK], F32, tag=tag, name=f"pb{h}")
                nc.tensor.matmul(out=pb[0:64, :], lhsT=ones64[HD:HD + 1, :],
                                 rhs=ent[1][HD:HD + 1, :],
                                 start=True, stop=True)
                rb = nrm.tile([64, TOK], BF, tag="rbb", name=f"rbb{h}")
                nc.vector.tensor_copy(out=rb[:, :], in_=pb[0:64, :])
                ent[2] = rb

            def norm_mul(p, j):
                h = pairs[p][j]
                ent = aph_of[(p, j)]
                t, half = h // 2, (h % 2) * 64
                if half == 0:
                    nc.vector.tensor_mul(out=attnT[t][0:64, :],
                                         in0=ent[0][0:HD, :],
                                         in1=ent[2][:, :])
                else:
                    ah = nrm.tile([64, TOK], BF, tag="ah", name=f"ah{h}")
                    nc.vector.tensor_mul(out=ah[:, :], in0=ent[0][0:HD, :],
                                         in1=ent[2][:, :])
                    nc.sync.dma_start(out=attnT[t][64:128, :], in_=ah[:, :])
                del aph_of[(p, j)]

            def norm_stage2(p, ci):
                if ci == 4:
                    norm_recip(p, 0)
                elif ci == 6:
                    norm_recip(p, 1)
                elif ci == 8:
                    norm_cast(p, 0)
                    norm_cast(p, 1)
                elif ci == 10:
                    norm_bcast(p, 0)
                elif ci == 12:
                    norm_bcast(p, 1)
                    norm_mul(p, 0)
                elif ci == 14:
                    norm_mul(p, 1)

            def emit_scores(pi, ci):
                hA, hB = pairs[pi]
                kt = (hA // GRP) // 2
                qiA, _ = q_slot(hA)
                qiB, _ = q_slot(hB)
                psA = sps.tile([128, 2 * TOK], F32, tag="ps",
                               name=f"psA{hA}_{ci}")
                psB = sps.tile([128, 2 * TOK], F32, tag="ps",
                               name=f"psB{hB}_{ci}")
                for dc in range(2):
                    c = ci + dc
                    nb, lc = c // 4, c % 4
                    kts = kTg[nb * 4 + kt]
                    nc.tensor.matmul(
                        out=psA[:, dc * TOK:(dc + 1) * TOK],
                        lhsT=kts[0:64, lc * 128:(lc + 1) * 128],
                        rhs=qTp[qiA][0:64, :], start=True, stop=True)
                ptA = ptp.tile([128, 2 * TOK], BF, tag="pt",
                               name=f"ptA{hA}_{ci}")
                nc.scalar.activation(
                    out=ptA[:, :], in_=psA[:, :],
                    func=mybir.ActivationFunctionType.Exp, scale=EXP_SCALE)
                for dc in range(2):
                    c = ci + dc
                    nb, lc = c // 4, c % 4
                    kts = kTg[nb * 4 + kt]
                    nc.tensor.matmul(
                        out=psB[:, dc * TOK:(dc + 1) * TOK],
                        lhsT=kts[64:128, lc * 128:(lc + 1) * 128],
                        rhs=qTp[qiB][64:128, :], start=True, stop=True)
                ptB = ptp.tile([128, 2 * TOK], BF, tag="pt",
                               name=f"ptB{hB}_{ci}")
                nc.scalar.activation(
                    out=ptB[:, :], in_=psB[:, :],
                    func=mybir.ActivationFunctionType.Exp, scale=EXP_SCALE)
                return ptA, ptB

            def emit_pv(pi, ci, ptA, ptB):
                hA, hB = pairs[pi]
                khA, khB = hA // GRP, hB // GRP
                poA, poB = po_of[pi]
                for dc in range(2):
                    c = ci + dc
                    nc.tensor.matmul(
                        out=poA[:, :],
                        lhsT=vg[c][:, khA * (HD + 1):(khA + 1) * (HD + 1)],
                        rhs=ptA[:, dc * TOK:(dc + 1) * TOK],
                        start=(c == 0), stop=(c == NKC - 1))
                    nc.tensor.matmul(
                        out=poB[:, :],
                        lhsT=vg[c][:, khB * (HD + 1):(khB + 1) * (HD + 1)],
                        rhs=ptB[:, dc * TOK:(dc + 1) * TOK],
                        start=(c == 0), stop=(c == NKC - 1))

            sched = [(pi, 2 * c2) for pi in range(len(pairs))
                     for c2 in range(NKC // 2)]
            po_of = {}
            inflight = deque()

            for s, (pi, ci) in enumerate(sched):
                hA, hB = pairs[pi]
                if ci == 0:
                    poA = ops.tile([HD + 1, TOK], F32, tag="po",
                                   name=f"poA{hA}")
                    poB = ops.tile([HD + 1, TOK], F32, tag="po",
                                   name=f"poB{hB}")
                    po_of[pi] = (poA, poB)
                # PV of step s-LAG first: it never waits, so it fills the
                # window where scores-A(s) stalls on exp(s-1) freeing the
                # score-PSUM ring (the PE queue is in-order).
                if len(inflight) >= LAG:
                    emit_pv(*inflight.popleft())
                ptA, ptB = emit_scores(pi, ci)
                inflight.append((pi, ci, ptA, ptB))
                if pi > 0:
                    if ci in (2, 4):
                        norm_copy(pi - 1, ci)
                    if ci >= 4:
                        norm_stage2(pi - 1, ci)
                # drip pops: real deadlines force correctness-critical
                # work; the adaptive quota spreads everything else so
                # the queue neither bursts (burying the next scores in
                # the in-order PE queue) nor runs dry (PE p-state drop).
                due = 0
                for dd, cc, _ in drip:
                    if dd > s + 25:
                        break
                    due += cc
                quota = max(rem_cost / max(1.0, 126.0 - s), due / 25.0)
                spent = 0
                while drip and (drip[0][0] <= s + 2 or spent < quota):
                    _, cost, th = drip.popleft()
                    th()
                    spent += cost
                    rem_cost -= cost
                # keep-warm fillers: an idle PE drops out of its max
                # p-state within ~1us and every matmul then runs ~1.6x
                # slow. On bare steps burn ~0.5us on K=1 junk matmuls.
                if spent < 400 and s > 4:
                    for f in range(2):
                        fp = dps.tile([128, TOK], F32, tag="dp",
                                      name=f"fil{s}_{f}")
                        nc.tensor.matmul(out=fp[:, :], lhsT=ones128[:, :],
                                         rhs=ones512[:, :],
                                         start=True, stop=True)
                # release xin at step 118 (all x/wk/wv consumers done by
                # ~110) and stream the Wo prefetch into the freed SBUF.
                if s == 118:
                    xin_cm.__exit__(None, None, None)
                    wo2_box.append(es.enter_context(
                        tc.tile_pool(name="wo2", bufs=1)))
                    for i, (cst, th) in enumerate(gen_wo_prefetch()):
                        drip.append((119 + i, cst, th))

            while inflight:
                emit_pv(*inflight.popleft())
            while drip:
                drip.popleft()[2]()

        # =============== phase E: output projection + bias ===========
        # sps/dps closed; ops stays open so pair 15's norm drain (which
        # reads po(15)) can overlap E's first 14 kc-groups — attnT[13]
        # and attnT[15] are the only pair-15-gated contraction chunks,
        # so they accumulate last.
        with tc.tile_pool(name="yps", bufs=6, space="PSUM") as yps, \
             tc.tile_pool(name="ystg", bufs=4) as ystg:
            bc_box[0] = (yps, "py")
            kc_order = list(range(13)) + [14, 13, 15]
            for nt in range(4):        # 4 output column blocks of 512
                wo_last = wo_pre[(nt, "last")]
                pys = [yps.tile([128, 512], F32, tag="py",
                                name=f"py{nt}_{i}") for i in range(4)]
                for idx, kc in enumerate(kc_order):
                    wo_t = wo_pre[(nt, kc)]
                    for mt in range(4):
                        nc.tensor.matmul(
                            out=pys[mt][:, :],
                            lhsT=attnT[kc][:, mt * 128:(mt + 1) * 128],
                            rhs=wo_t[:, :],
                            start=(idx == 0), stop=False)
                    if nt == 0 and idx == 13:
                        # pair-15 norm drain: DVE chain runs while the
                        # PE streams the kc-groups emitted above
                        for ci in (2, 4):
                            norm_copy(15, ci)
                        for ci in range(4, 16, 2):
                            norm_stage2(15, ci)
                for mt in range(4):    # bias via ones row, K=1 matmul
                    nc.tensor.matmul(
                        out=pys[mt][:, :], lhsT=ones128[:, :],
                        rhs=wo_last[:, :], start=False, stop=True)
                    ys = ystg.tile([128, 512], F32, tag="ys",
                                   name=f"ys{nt}_{mt}")
                    nc.vector.tensor_copy(out=ys[:, :], in_=pys[mt][:, :])
                    nc.sync.dma_start(
                        out=out[mt * 128:(mt + 1) * 128,
                                nt * 512:(nt + 1) * 512],
                        in_=ys[:, :])
        ops_cm.__exit__(None, None, None)

    nc.finalize()
    return nc


@functools.lru_cache(maxsize=1)
def _graph():
    return build_graph()


def make_in_maps(x, Wq, Wk, Wv, Wo, bo):
    bf16 = ml_dtypes.bfloat16
    x = np.asarray(x, np.float32)
    wqT = np.asarray(Wq, np.float32).T                    # [HID, HID]
    # pre-tiled so one [128, KC*128] DMA loads a whole Q chunk's weights
    wqTk = np.ascontiguousarray(
        wqT.reshape(KC, 128, HID).transpose(1, 0, 2)).astype(bf16)
    wkvT = np.ascontiguousarray(np.concatenate(
        [np.asarray(Wk, np.float32).T, np.asarray(Wv, np.float32).T],
        axis=1)).astype(bf16)                             # [HID, 1024]
    woT = np.concatenate(
        [np.asarray(Wo, np.float32).T,
         np.asarray(bo, np.float32)[None, :]], axis=0).astype(bf16)
    woT = np.ascontiguousarray(woT)
    in_maps = []
    for c in range(8):
        b, r = c // TP, c % TP
        # token permutation: own query block first, rest after (attention
        # is permutation-invariant over keys)
        perm = np.r_[r * TOK:(r + 1) * TOK, 0:r * TOK, (r + 1) * TOK:S]
        xT_c = np.ascontiguousarray(x[b].T[:, perm]).astype(bf16)
        in_maps.append(
            {"xT": xT_c, "wkvT": wkvT, "wqTk": wqTk, "woT": woT})
    return in_maps


def kernel(x, Wq, Wk, Wv, Wo, bo):
    nc = _graph()
    in_maps = make_in_maps(x, Wq, Wk, Wv, Wo, bo)
    res = run_bass_kernel_spmd(nc, in_maps, core_ids=list(range(8)))
    out = np.empty((B, S, HID), np.float32)
    for c in range(8):
        b, r = c // TP, c % TP
        out[b, r * TOK:(r + 1) * TOK, :] = np.asarray(
            res.results[c]["out"], np.float32)
    return out


# revision 37
# speedup vs baseline: 1.3281x; 1.0052x over previous
"""GQA attention (B=2, S=2048, HID=2048, 32 q heads / 8 kv heads, fp32 I/O)
on 8 TRN2 NeuronCores.

Sharding: sequence-parallel with fully local K/V. Core c owns 512 query
tokens of batch c//4 (cores 0-3 = batch 0, cores 4-7 = batch 1), but
computes K^T and V for ALL 2048 tokens of its batch locally — that
(+~40% KV projection FLOPs) is much cheaper than an intra-chip
AllGather, which measures 100-170us and blockades the DMA engines while
it runs. Attention is permutation-invariant over keys, so each core
orders tokens own-block-first (host-side permutation) and the device
program stays rank-independent.

V carries a fused ones-column per kv head so the PV matmul also
produces the softmax row-sums; the output-projection bias is fused as
an extra contraction row. All matmuls run in bf16 with fp32 PSUM
accumulation. Score matmuls (K=64) pair head A (partitions 0-63) and
head B (64-127) on disjoint PE row-groups so the hardware co-executes
them (auto tile_position from base partitions).

v2 schedule: the serial K/V-projection prologue is collapsed to one
8-bank PSUM wave {K mt0 x4, Q0, Q2, V(c0,kv0-3), V(c1,kv0-3)} streamed
chunk-outer against the x DMA arrival, so the first exp fires at
~45us (was ~122us). Everything else — V in 30 finer (chunk, kv-half)
units, K mt1-mt3, Q chunks, Wo prefetch — drips into the ACT-bound
pair loop through a 2-bank ping-pong PSUM ring with deadline-forced,
cost-budgeted pops. The xin pool (x remainder + Wk/Wv) releases at
step 96 to make room for prefetching all 4 Wo column blocks, so the
output projection runs as a pure-PE tail with all 8 PSUM banks.
"""

import functools
from collections import deque
from contextlib import ExitStack

import numpy as np
import ml_dtypes

import concourse.bass as bass
import concourse.mybir as mybir
import concourse.tile as tile
from concourse import bacc
from concourse.bass_utils import run_bass_kernel_spmd

BF = mybir.dt.bfloat16
F32 = mybir.dt.float32

B, S, HID = 2, 2048, 2048
NH, NKV, HD = 32, 8, 64          # q heads, kv heads, head dim
GRP = NH // NKV                  # 4 q heads per kv head
TP = 4                           # cores per batch group
TOK = S // TP                    # 512 local query tokens per core
KC = HID // 128                  # 16 contraction chunks of 128
NKC = S // 128                   # 16 key chunks of 128 (full seq)
VW = NKV * (HD + 1)              # 520: V width incl. ones columns
EXP_SCALE = float(HD) ** -0.5    # 1/8 softmax scale, fused into Exp
LAG = 2                          # steps between scores+exp and its PV


def q_slot(h):
    """qTp tile index and partition base for head h.

    Head h lives at partition base ((h//4)%2)*64 — the same base its kv
    head kh=h//4 occupies inside the kTg tiles, so the scores matmul's
    lhsT and rhs stay partition-aligned (and heads A/B co-execute on
    disjoint PE row groups).
    """
    return ((h // 4) // 2) * 4 + (h % 4), ((h // 4) % 2) * 64


def build_graph():
    nc = bacc.Bacc(None, target_bir_lowering=False, debug=False, num_devices=8)

    # DMA issue slots on the sync queue cost ~650ns EACH regardless of
    # size, so inputs are host-packed for one-issue-per-tile transfers:
    # wkvT = Wk^T|Wv^T fused, wqTk = Wq^T pre-tiled so a whole Q-chunk's
    # 16 weight tiles land in one [128, 16*128] DMA.
    xT = nc.declare_dram_parameter("xT", [HID, S], BF, isOutput=False)
    wkvT = nc.declare_dram_parameter("wkvT", [HID, 2 * NKV * HD], BF,
                                     isOutput=False)
    wqTk = nc.declare_dram_parameter("wqTk", [128, KC, HID], BF,
                                     isOutput=False)
    woT = nc.declare_dram_parameter("woT", [HID + 1, HID], BF, isOutput=False)
    out = nc.declare_dram_parameter("out", [TOK, HID], F32, isOutput=True)

    with tile.TileContext(nc) as tc, ExitStack() as es:
        pers = es.enter_context(tc.tile_pool(name="pers", bufs=1))

        def T(shape, dtype, *, name):
            return pers.tile(shape, dtype, name=name, tag=name)

        # long-lived SBUF pools first; xin LAST so it can release at
        # step 96 while still top-of-stack among SBUF pools.
        wqp = es.enter_context(tc.tile_pool(name="wqp", bufs=4))
        stgB = es.enter_context(tc.tile_pool(name="stgB", bufs=2))
        nrm = es.enter_context(tc.tile_pool(name="nrm", bufs=2))
        ptp = es.enter_context(tc.tile_pool(name="ptp", bufs=6))

        xin_cm = tc.tile_pool(name="xin", bufs=1)
        xin = xin_cm.__enter__()
        xf = [xin.tile([128, S], BF, tag=f"xf{k}", name=f"xf{k}")
              for k in range(KC)]
        wkv = [xin.tile([128, 2 * NKV * HD], BF, tag=f"wkv{k}", name=f"wkv{k}")
               for k in range(KC)]

        def wk_col(k, lo, n):
            return wkv[k][:, lo:lo + n]

        def wv_col(k, lo, n):
            return wkv[k][:, 512 + lo:512 + lo + n]

        # whole-unit weight tiles for wave-0's Q0/Q2
        wq_w0 = {m: wqp.tile([128, KC * 128], BF, tag="wq", name=f"wqw{m}")
                 for m in (0, 2)}

        # ---- DMA issue order = priority. Two issues per x chunk; wq
        # whole-unit tiles first so wave 0's Q matmuls never stall the
        # in-order PE queue.
        for m in (0, 2):
            nc.sync.dma_start(out=wq_w0[m][:, :],
                              in_=wqTk[:, :, m * 128:(m + 1) * 128])
        for k in range(KC):
            nc.sync.dma_start(out=wkv[k][:, :],
                              in_=wkvT[k * 128:(k + 1) * 128, :])
            nc.sync.dma_start(out=xf[k][:, :], in_=xT[k * 128:(k + 1) * 128, :])

        def xcols(k, lo, n):
            # columns lo..lo+n of the permuted x^T chunk k
            return xf[k][:, lo:lo + n]

        # row HD (partition 64) is the K=1 lhsT for the row-sum broadcast
        ones64 = T([HD + 1, 64], BF, name="ones64")
        nc.vector.memset(ones64[:, :], 1.0)
        ones128 = T([1, 128], BF, name="ones128")
        nc.vector.memset(ones128[:, :], 1.0)
        ones512 = T([1, TOK], BF, name="ones512")
        nc.vector.memset(ones512[:, :], 1.0)

        # kTg[nb*4+mt]: [128, 512] = K^T rows mt*128.. for key block nb
        # (kv heads 2mt at partitions 0-63, 2mt+1 at 64-127).
        # vg[c]: [128, 520] V_aug rows for key chunk c, ones at col
        # kh*65+64 of each kv head kh.
        kTg = [T([128, TOK], BF, name=f"kTg{i}") for i in range(16)]
        vg = [T([128, VW], BF, name=f"vg{c}") for c in range(NKC)]
        qTp = [T([128, TOK], BF, name=f"qTp{i}") for i in range(NH // 2)]
        attnT = [T([128, TOK], BF, name=f"attnT{t}") for t in range(NH // 2)]

        def q_fin(ps, m):
            st = stgB.tile([128, TOK], BF, tag="stg", name=f"stq{m}")
            nc.vector.tensor_copy(out=st[:, :], in_=ps[:, :])
            for j in range(2):
                h = 2 * m + j
                i, roff = q_slot(h)
                nc.sync.dma_start(out=qTp[i][roff:roff + 64, :],
                                  in_=st[j * 64:(j + 1) * 64, :])

        def v_fin(ps, c, h):
            # single strided copy (4 tiny copies would each pay the
            # ~0.2us DVE drain bubble and clog the in-order DVE queue)
            if h == 0:
                nc.vector.memset(vg[c][:, :], 1.0)
            dst = vg[c][:, 4 * h * (HD + 1):(4 * h + 4) * (HD + 1)]
            nc.vector.tensor_copy(
                out=dst.rearrange("p (g c) -> p g c", g=4)[:, :, 0:HD],
                in_=ps.rearrange("p (g c) -> p g c", g=4))

        # =============== wave 0: the minimal exp-gating work ============
        # 8 PSUM accumulation groups, contraction-chunk OUTER so the PE
        # streams 8 matmuls per arriving x chunk. Completes ~1.7us after
        # the last x chunk lands; first exp fires ~2us later.
        w0 = ([("k", 0, nb) for nb in range(TP)]
              + [("q", 0, None), ("q", 2, None)]
              + [("v", 0, 0), ("v", 1, 0)])
        with tc.tile_pool(name="accA", bufs=8, space="PSUM") as accA:
            pss = [accA.tile([128, TOK], F32, tag="acc", name=f"psA{i}")
                   for i in range(8)]
            for k in range(KC):
                for ps, (kind, a, b) in zip(pss, w0):
                    if kind == "k":
                        nc.tensor.matmul(
                            out=ps[:, :],
                            lhsT=wk_col(k, 0, 128),
                            rhs=xcols(k, b * TOK, TOK),
                            start=(k == 0), stop=(k == KC - 1))
                    elif kind == "q":
                        nc.tensor.matmul(
                            out=ps[:, :],
                            lhsT=wq_w0[a][:, k * 128:(k + 1) * 128],
                            rhs=xcols(k, 0, TOK),
                            start=(k == 0), stop=(k == KC - 1))
                    else:
                        nc.tensor.matmul(
                            out=ps[:, 0:256],
                            lhsT=xcols(k, a * 128, 128),
                            rhs=wv_col(k, 0, 256),
                            start=(k == 0), stop=(k == KC - 1))
            # evac in exp-gating order: kTg nb0, Q0, Q2, then the rest
            nc.vector.tensor_copy(out=kTg[0][:, :], in_=pss[0][:, :])
            q_fin(pss[4], 0)
            q_fin(pss[5], 2)
            for nb in range(1, TP):
                nc.vector.tensor_copy(out=kTg[nb * 4][:, :],
                                      in_=pss[nb][:, :])
            v_fin(pss[6][:, 0:256], 0, 0)
            v_fin(pss[7][:, 0:256], 1, 0)

        # =============== drip units (fed into the pair loop) ============
        # Each unit: (deadline_step, [(cost_ns, thunk), ...]).
        MM_NS = 216    # 512-col bf16 matmul streaming time
        VMM_NS = 112   # 256-col

        drip = deque()  # (deadline, cost_ns, thunk)
        _units = []     # (deadline, seq, [(cost, thunk), ...])

        def push_unit(deadline, cts):
            _units.append((deadline, len(_units), cts))

        def seal_units():
            # stable-sort by real deadline; pacing happens at pop time
            # via an adaptive quota (see the pair loop)
            total = 0
            for d, _, cts in sorted(_units, key=lambda u: (u[0], u[1])):
                for c, t in cts:
                    drip.append((d, c, t))
                    total += c
            return total

        # unit PSUM tiles MUST allocate lazily at pop time: the dps ring
        # orders its WAR handoffs by .tile() call order, which has to
        # match emission order (norm_bcast's pb tiles share the ring).
        def gen_q_chunk(m):
            """one whole-unit weight DMA, 16 matmuls, finisher."""
            box = {}
            ws = wqp.tile([128, KC * 128], BF, tag="wq", name=f"wqu{m}")
            def dm(m=m):
                nc.sync.dma_start(out=ws[:, :],
                                  in_=wqTk[:, :, m * 128:(m + 1) * 128])
            thunks = [(0, dm)]
            for k in range(KC):
                def mm(k=k, m=m):
                    if k == 0:
                        box["ps"] = dps.tile([128, TOK], F32, tag="dp",
                                             name=f"psq{m}")
                    nc.tensor.matmul(
                        out=box["ps"][:, :],
                        lhsT=ws[:, k * 128:(k + 1) * 128],
                        rhs=xcols(k, 0, TOK),
                        start=(k == 0), stop=(k == KC - 1))
                thunks.append((MM_NS, mm))
            thunks.append((0, lambda m=m: q_fin(box["ps"], m)))
            return thunks

        def gen_k_unit(mt, nb):
            box = {}
            thunks = []
            for k in range(KC):
                def mm(k=k, mt=mt, nb=nb):
                    if k == 0:
                        box["ps"] = dps.tile([128, TOK], F32, tag="dp",
                                             name=f"psk{nb}_{mt}")
                    nc.tensor.matmul(
                        out=box["ps"][:, :],
                        lhsT=wk_col(k, mt * 128, 128),
                        rhs=xcols(k, nb * TOK, TOK),
                        start=(k == 0), stop=(k == KC - 1))
                thunks.append((MM_NS, mm))
            def fin(mt=mt, nb=nb):
                nc.vector.tensor_copy(out=kTg[nb * 4 + mt][:, :],
                                      in_=box["ps"][:, :])
            thunks.append((0, fin))
            return thunks

        def gen_v_unit(c, h):
            box = {}
            thunks = []
            for k in range(KC):
                def mm(k=k, c=c, h=h):
                    if k == 0:
                        box["ps"] = dps.tile([128, TOK], F32, tag="dp",
                                             name=f"psv{c}_{h}")
                    nc.tensor.matmul(
                        out=box["ps"][:, 0:256],
                        lhsT=xcols(k, c * 128, 128),
                        rhs=wv_col(k, h * 256, 256),
                        start=(k == 0), stop=(k == KC - 1))
                thunks.append((VMM_NS, mm))
            thunks.append(
                (0, lambda c=c, h=h: v_fin(box["ps"][:, 0:256], c, h)))
            return thunks

        # Wo prefetch (DMA-only): queued when xin releases at step 118.
        # One [128, 2048] row-block DMA per kc (all 4 nt at once).
        wo_pre = {}
        wo2_box = []

        def gen_wo_prefetch():
            thunks = []
            def last():
                wl = wo2_box[0].tile([1, HID], BF, tag="wolast",
                                     bufs=1, name="wl")
                nc.sync.dma_start(out=wl[:, :], in_=woT[HID:HID + 1, :])
                for nt in range(4):
                    wo_pre[(nt, "last")] = wl[:, nt * 512:(nt + 1) * 512]
            thunks.append((0, last))
            for kc in range(KC):
                def f(kc=kc):
                    w = wo2_box[0].tile([128, HID], BF, tag="wo", bufs=16,
                                        name=f"wo{kc}")
                    nc.sync.dma_start(
                        out=w[:, :], in_=woT[kc * 128:(kc + 1) * 128, :])
                    for nt in range(4):
                        wo_pre[(nt, kc)] = w[:, nt * 512:(nt + 1) * 512]
                thunks.append((0, f))
            return thunks

        # =============== the pair loop ==================================
        pairs = []
        for g in range(0, NKV, 2):
            for j in range(GRP):
                pairs.append((g * GRP + j, (g + 1) * GRP + j))

        bc_box = [None]  # (pool, tag) for the row-sum broadcast PSUM
        ops_cm = tc.tile_pool(name="ops", bufs=2, space="PSUM")
        ops = ops_cm.__enter__()
        with tc.tile_pool(name="sps", bufs=2, space="PSUM") as sps, \
             tc.tile_pool(name="dps", bufs=2, space="PSUM") as dps:
            bc_box[0] = (dps, "dp")

            # drip queue in deadline order
            def push_q(d, m):
                ths = gen_q_chunk(m)
                # weight DMA leads its matmuls by ~4 steps so the 512KB
                # transfer never head-of-line blocks the PE queue
                push_unit(max(0, d - 4), [ths[0]])
                push_unit(d, ths[1:])

            for c in range(2, NKC):                      # V kv0-3 rest
                push_unit(c // 2 + 1, gen_v_unit(c, 0))
            push_q(13, 1)
            push_q(13, 3)
            for nb in range(TP):                         # K mt1
                push_unit(28 + 2 * nb, gen_k_unit(1, nb))
            push_q(29, 4)
            push_q(29, 6)
            push_q(44, 5)
            push_q(44, 7)
            for c in range(NKC):                         # V kv4-7
                push_unit(61 + c // 2, gen_v_unit(c, 1))
            for nb in range(TP):                         # K mt2
                push_unit(61 + 2 * nb, gen_k_unit(2, nb))
            push_q(61, 8)
            push_q(61, 10)
            push_q(76, 9)
            push_q(76, 11)
            for nb in range(TP):                         # K mt3
                push_unit(92 + 2 * nb, gen_k_unit(3, nb))
            push_q(93, 12)
            push_q(93, 14)
            push_q(105, 13)
            push_q(105, 15)
            rem_cost = seal_units()

            # Normalization for pair p staged across pair p+1's steps
            # (PSUM->SBUF copy, reciprocal, cast, PE ones-broadcast via
            # the dps ring, multiply) so the 3.3us DVE reciprocal never
            # blocks a PE-side consumer.
            aph_of = {}

            def norm_copy(p, ci):
                j = 0 if ci == 2 else 1
                h = pairs[p][j]
                po = po_of[p][j]
                aph = nrm.tile([HD + 1, TOK], F32, tag="aph", bufs=2,
                               name=f"aph{h}")
                nc.vector.tensor_copy(out=aph[:, :], in_=po[:, :])
                aph_of[(p, j)] = [aph, None, None]
                if ci == 4:
                    del po_of[p]

            def norm_recip(p, j):
                # DVE reciprocal is an iterative divide, and the row sum
                # lives on ONE partition: [1,512] costs 3.3us on a single
                # lane. Spread it to [128,4] via a reshaping SBUF->SBUF
                # DMA so all 128 lanes divide in parallel (~0.15us).
                h = pairs[p][j]
                ent = aph_of[(p, j)]
                z4 = nrm.tile([128, 4], F32, tag="z4", name=f"z4{h}")
                nc.sync.dma_start(out=z4[:, :], in_=ent[0][HD:HD + 1, :])
                r4 = nrm.tile([128, 4], F32, tag="r4", name=f"r4{h}")
                nc.vector.reciprocal(out=r4[:, :], in_=z4[:, :])
                rb4 = nrm.tile([128, 4], BF, tag="rb4", name=f"rb4{h}")
                nc.vector.tensor_copy(out=rb4[:, :], in_=r4[:, :])
                ent[1] = rb4

            def norm_cast(p, j):
                # gather the spread reciprocal back to [1,512] bf16
                h = pairs[p][j]
                ent = aph_of[(p, j)]
                rcpb = nrm.tile([HD + 1, TOK], BF, tag="rcpb", name=f"rb{h}")
                nc.sync.dma_start(out=rcpb[HD:HD + 1, :], in_=ent[1][:, :])
                ent[1] = rcpb

            def norm_bcast(p, j):
                h = pairs[p][j]
                ent = aph_of[(p, j)]
                pool, tag = bc_box[0]
                pb = pool.tile([128, TOK], F32, tag=tag, name=f"pb{h}")
                nc.tensor.matmul(out=pb[0:64, :], lhsT=ones64[HD:HD + 1, :],
                                 rhs=ent[1][HD:HD + 1, :],
                                 start=True, stop=True)
                rb = nrm.tile([64, TOK], BF, tag="rbb", name=f"rbb{h}")
                nc.vector.tensor_copy(out=rb[:, :], in_=pb[0:64, :])
                ent[2] = rb

            def norm_mul(p, j):
                h = pairs[p][j]
                ent = aph_of[(p, j)]
                t, half = h // 2, (h % 2) * 64
                if half == 0:
                    nc.vector.tensor_mul(out=attnT[t][0:64, :],
                                         in0=ent[0][0:HD, :],
                                         in1=ent[2][:, :])
                else:
                    ah = nrm.tile([64, TOK], BF, tag="ah", name=f"ah{h}")
                    nc.vector.tensor_mul(out=ah[:, :], in0=ent[0][0:HD, :],
                                         in1=ent[2][:, :])
                    nc.sync.dma_start(out=attnT[t][64:128, :], in_=ah[:, :])
                del aph_of[(p, j)]

            def norm_stage2(p, ci):
                if ci == 4:
                    norm_recip(p, 0)
                elif ci == 6:
                    norm_recip(p, 1)
                elif ci == 8:
                    norm_cast(p, 0)
                    norm_cast(p, 1)
                elif ci == 10:
                    norm_bcast(p, 0)
                elif ci == 12:
                    norm_bcast(p, 1)
                    norm_mul(p, 0)
                elif ci == 14:
                    norm_mul(p, 1)

            def emit_scores(pi, ci):
                hA, hB = pairs[pi]
                kt = (hA // GRP) // 2
                qiA, _ = q_slot(hA)
                qiB, _ = q_slot(hB)
                psA = sps.tile([128, 2 * TOK], F32, tag="ps",
                               name=f"psA{hA}_{ci}")
                psB = sps.tile([128, 2 * TOK], F32, tag="ps",
                               name=f"psB{hB}_{ci}")
                for dc in range(2):
                    c = ci + dc
                    nb, lc = c // 4, c % 4
                    kts = kTg[nb * 4 + kt]
                    nc.tensor.matmul(
                        out=psA[:, dc * TOK:(dc + 1) * TOK],
                        lhsT=kts[0:64, lc * 128:(lc + 1) * 128],
                        rhs=qTp[qiA][0:64, :], start=True, stop=True)
                ptA = ptp.tile([128, 2 * TOK], BF, tag="pt",
                               name=f"ptA{hA}_{ci}")
                nc.scalar.activation(
                    out=ptA[:, :], in_=psA[:, :],
                    func=mybir.ActivationFunctionType.Exp, scale=EXP_SCALE)
                for dc in range(2):
                    c = ci + dc
                    nb, lc = c // 4, c % 4
                    kts = kTg[nb * 4 + kt]
                    nc.tensor.matmul(
                        out=psB[:, dc * TOK:(dc + 1) * TOK],
                        lhsT=kts[64:128, lc * 128:(lc + 1) * 128],
                        rhs=qTp[qiB][64:128, :], start=True, stop=True)
                ptB = ptp.tile([128, 2 * TOK], BF, tag="pt",
                               name=f"ptB{hB}_{ci}")
                nc.scalar.activation(
                    out=ptB[:, :], in_=psB[:, :],
                    func=mybir.ActivationFunctionType.Exp, scale=EXP_SCALE)
                return ptA, ptB

            def emit_pv(pi, ci, ptA, ptB):
                hA, hB = pairs[pi]
                khA, khB = hA // GRP, hB // GRP
                poA, poB = po_of[pi]
                for dc in range(2):
                    c = ci + dc
                    nc.tensor.matmul(
                        out=poA[:, :],
                        lhsT=vg[c][:, khA * (HD + 1):(khA + 1) * (HD + 1)],
                        rhs=ptA[:, dc * TOK:(dc + 1) * TOK],
                        start=(c == 0), stop=(c == NKC - 1))
                    nc.tensor.matmul(
                        out=poB[:, :],
                        lhsT=vg[c][:, khB * (HD + 1):(khB + 1) * (HD + 1)],
                        rhs=ptB[:, dc * TOK:(dc + 1) * TOK],
                        start=(c == 0), stop=(c == NKC - 1))

            sched = [(pi, 2 * c2) for pi in range(len(pairs))
                     for c2 in range(NKC // 2)]
            po_of = {}
            inflight = deque()

            for s, (pi, ci) in enumerate(sched):
                hA, hB = pairs[pi]
                if ci == 0:
                    poA = ops.tile([HD + 1, TOK], F32, tag="po",
                                   name=f"poA{hA}")
                    poB = ops.tile([HD + 1, TOK], F32, tag="po",
                                   name=f"poB{hB}")
                    po_of[pi] = (poA, poB)
                # PV of step s-LAG first: it never waits, so it fills the
                # window where scores-A(s) stalls on exp(s-1) freeing the
                # score-PSUM ring (the PE queue is in-order).
                if len(inflight) >= LAG:
                    emit_pv(*inflight.popleft())
                ptA, ptB = emit_scores(pi, ci)
                inflight.append((pi, ci, ptA, ptB))
                if pi > 0:
                    if ci in (2, 4):
                        norm_copy(pi - 1, ci)
                    if ci >= 4:
                        norm_stage2(pi - 1, ci)
                # drip pops: real deadlines force correctness-critical
                # work; the adaptive quota spreads everything else so
                # the queue neither bursts (burying the next scores in
                # the in-order PE queue) nor runs dry (PE p-state drop).
                due = 0
                for dd, cc, _ in drip:
                    if dd > s + 25:
                        break
                    due += cc
                quota = max(rem_cost / max(1.0, 126.0 - s), due / 25.0)
                spent = 0
                while drip and (drip[0][0] <= s + 2 or spent < quota):
                    _, cost, th = drip.popleft()
                    th()
                    spent += cost
                    rem_cost -= cost
                # keep-warm fillers: an idle PE drops out of its max
                # p-state within ~1us and every matmul then runs ~1.6x
                # slow. On bare steps burn ~0.5us on K=1 junk matmuls.
                if spent < 400 and s > 4:
                    for f in range(2):
                        fp = dps.tile([128, TOK], F32, tag="dp",
                                      name=f"fil{s}_{f}")
                        nc.tensor.matmul(out=fp[:, :], lhsT=ones128[:, :],
                                         rhs=ones512[:, :],
                                         start=True, stop=True)
                # release xin at step 118 (all x/wk/wv consumers done by
                # ~110) and stream the Wo prefetch into the freed SBUF.
                if s == 118:
                    xin_cm.__exit__(None, None, None)
                    wo2_box.append(es.enter_context(
                        tc.tile_pool(name="wo2", bufs=1)))
                    for i, (cst, th) in enumerate(gen_wo_prefetch()):
                        drip.append((119 + i, cst, th))

            while inflight:
                emit_pv(*inflight.popleft())
            while drip:
                drip.popleft()[2]()

        # =============== phase E: output projection + bias ===========
        # sps/dps closed; ops stays open so pair 15's norm drain (which
        # reads po(15)) can overlap E's first 14 kc-groups — attnT[13]
        # and attnT[15] are the only pair-15-gated contraction chunks,
        # so they accumulate last.
        with tc.tile_pool(name="yps", bufs=6, space="PSUM") as yps, \
             tc.tile_pool(name="ystg", bufs=4) as ystg:
            bc_box[0] = (yps, "py")
            kc_order = list(range(13)) + [14, 13, 15]
            for nt in range(4):        # 4 output column blocks of 512
                wo_last = wo_pre[(nt, "last")]
                pys = [yps.tile([128, 512], F32, tag="py",
                                name=f"py{nt}_{i}") for i in range(4)]
                for idx, kc in enumerate(kc_order):
                    wo_t = wo_pre[(nt, kc)]
                    for mt in range(4):
                        nc.tensor.matmul(
                            out=pys[mt][:, :],
                            lhsT=attnT[kc][:, mt * 128:(mt + 1) * 128],
                            rhs=wo_t[:, :],
                            start=(idx == 0), stop=False)
                    if nt == 0 and idx == 13:
                        # pair-15 norm drain: DVE chain runs while the
                        # PE streams the kc-groups emitted above
                        for ci in (2, 4):
                            norm_copy(15, ci)
                        for ci in range(4, 16, 2):
                            norm_stage2(15, ci)
                for mt in range(4):    # bias via ones row, K=1 matmul
                    nc.tensor.matmul(
                        out=pys[mt][:, :], lhsT=ones128[:, :],
                        rhs=wo_last[:, :], start=False, stop=True)
                    ys = ystg.tile([128, 512], F32, tag="ys",
                                   name=f"ys{nt}_{mt}")
                    nc.vector.tensor_copy(out=ys[:, :], in_=pys[mt][:, :])
                    nc.sync.dma_start(
                        out=out[mt * 128:(mt + 1) * 128,
                                nt * 512:(nt + 1) * 512],
                        in_=ys[:, :])
        ops_cm.__exit__(None, None, None)

    nc.finalize()
    return nc


@functools.lru_cache(maxsize=1)
def _graph():
    return build_graph()


def make_in_maps(x, Wq, Wk, Wv, Wo, bo):
    bf16 = ml_dtypes.bfloat16
    x = np.asarray(x, np.float32)
    wqT = np.asarray(Wq, np.float32).T                    # [HID, HID]
    # pre-tiled so one [128, KC*128] DMA loads a whole Q chunk's weights
    wqTk = np.ascontiguousarray(
        wqT.reshape(KC, 128, HID).transpose(1, 0, 2)).astype(bf16)
    wkvT = np.ascontiguousarray(np.concatenate(
        [np.asarray(Wk, np.float32).T, np.asarray(Wv, np.float32).T],
        axis=1)).astype(bf16)                             # [HID, 1024]
    woT = np.concatenate(
        [np.asarray(Wo, np.float32).T,
         np.asarray(bo, np.float32)[None, :]], axis=0).astype(bf16)
    woT = np.ascontiguousarray(woT)
    in_maps = []
    for c in range(8):
        b, r = c // TP, c % TP
        # token permutation: own query block first, rest after (attention
        # is permutation-invariant over keys)
        perm = np.r_[r * TOK:(r + 1) * TOK, 0:r * TOK, (r + 1) * TOK:S]
        xT_c = np.ascontiguousarray(x[b].T[:, perm]).astype(bf16)
        in_maps.append(
            {"xT": xT_c, "wkvT": wkvT, "wqTk": wqTk, "woT": woT})
    return in_maps


def kernel(x, Wq, Wk, Wv, Wo, bo):
    nc = _graph()
    in_maps = make_in_maps(x, Wq, Wk, Wv, Wo, bo)
    res = run_bass_kernel_spmd(nc, in_maps, core_ids=list(range(8)))
    out = np.empty((B, S, HID), np.float32)
    for c in range(8):
        b, r = c // TP, c % TP
        out[b, r * TOK:(r + 1) * TOK, :] = np.asarray(
            res.results[c]["out"], np.float32)
    return out


# revision 40
# speedup vs baseline: 1.3336x; 1.0042x over previous
"""GQA attention (B=2, S=2048, HID=2048, 32 q heads / 8 kv heads, fp32 I/O)
on 8 TRN2 NeuronCores.

Sharding: sequence-parallel with fully local K/V. Core c owns 512 query
tokens of batch c//4 (cores 0-3 = batch 0, cores 4-7 = batch 1), but
computes K^T and V for ALL 2048 tokens of its batch locally — that
(+~40% KV projection FLOPs) is much cheaper than an intra-chip
AllGather, which measures 100-170us and blockades the DMA engines while
it runs. Attention is permutation-invariant over keys, so each core
orders tokens own-block-first (host-side permutation) and the device
program stays rank-independent.

V carries a fused ones-column per kv head so the PV matmul also
produces the softmax row-sums; the output-projection bias is fused as
an extra contraction row. All matmuls run in bf16 with fp32 PSUM
accumulation. Score matmuls (K=64) pair head A (partitions 0-63) and
head B (64-127) on disjoint PE row-groups so the hardware co-executes
them (auto tile_position from base partitions).

v2 schedule: the serial K/V-projection prologue is collapsed to one
8-bank PSUM wave {K mt0 x4, Q0, Q2, V(c0,kv0-3), V(c1,kv0-3)} streamed
chunk-outer against the x DMA arrival, so the first exp fires at
~45us (was ~122us). Everything else — V in 30 finer (chunk, kv-half)
units, K mt1-mt3, Q chunks, Wo prefetch — drips into the ACT-bound
pair loop through a 2-bank ping-pong PSUM ring with deadline-forced,
cost-budgeted pops. The xin pool (x remainder + Wk/Wv) releases at
step 96 to make room for prefetching all 4 Wo column blocks, so the
output projection runs as a pure-PE tail with all 8 PSUM banks.
"""

import functools
from collections import deque
from contextlib import ExitStack

import numpy as np
import ml_dtypes

import concourse.bass as bass
import concourse.mybir as mybir
import concourse.tile as tile
from concourse import bacc
from concourse.bass_utils import run_bass_kernel_spmd

BF = mybir.dt.bfloat16
F32 = mybir.dt.float32

B, S, HID = 2, 2048, 2048
NH, NKV, HD = 32, 8, 64          # q heads, kv heads, head dim
GRP = NH // NKV                  # 4 q heads per kv head
TP = 4                           # cores per batch group
TOK = S // TP                    # 512 local query tokens per core
KC = HID // 128                  # 16 contraction chunks of 128
NKC = S // 128                   # 16 key chunks of 128 (full seq)
VW = NKV * (HD + 1)              # 520: V width incl. ones columns
EXP_SCALE = float(HD) ** -0.5    # 1/8 softmax scale, fused into Exp
LAG = 2                          # steps between scores+exp and its PV


def q_slot(h):
    """qTp tile index and partition base for head h.

    Head h lives at partition base ((h//4)%2)*64 — the same base its kv
    head kh=h//4 occupies inside the kTg tiles, so the scores matmul's
    lhsT and rhs stay partition-aligned (and heads A/B co-execute on
    disjoint PE row groups).
    """
    return ((h // 4) // 2) * 4 + (h % 4), ((h // 4) % 2) * 64


def build_graph():
    nc = bacc.Bacc(None, target_bir_lowering=False, debug=False, num_devices=8)

    # DMA issue slots on the sync queue cost ~650ns EACH regardless of
    # size, so inputs are host-packed for one-issue-per-tile transfers:
    # wkvT = Wk^T|Wv^T fused, wqTk = Wq^T pre-tiled so a whole Q-chunk's
    # 16 weight tiles land in one [128, 16*128] DMA.
    xT = nc.declare_dram_parameter("xT", [HID, S], BF, isOutput=False)
    wkvT = nc.declare_dram_parameter("wkvT", [HID, 2 * NKV * HD], BF,
                                     isOutput=False)
    wqTk = nc.declare_dram_parameter("wqTk", [128, KC, HID], BF,
                                     isOutput=False)
    woT = nc.declare_dram_parameter("woT", [HID + 1, HID], BF, isOutput=False)
    out = nc.declare_dram_parameter("out", [TOK, HID], F32, isOutput=True)

    with tile.TileContext(nc) as tc, ExitStack() as es:
        pers = es.enter_context(tc.tile_pool(name="pers", bufs=1))

        def T(shape, dtype, *, name):
            return pers.tile(shape, dtype, name=name, tag=name)

        # long-lived SBUF pools first; xin LAST so it can release at
        # step 96 while still top-of-stack among SBUF pools.
        wqp = es.enter_context(tc.tile_pool(name="wqp", bufs=4))
        stgB = es.enter_context(tc.tile_pool(name="stgB", bufs=2))
        nrm = es.enter_context(tc.tile_pool(name="nrm", bufs=2))
        ptp = es.enter_context(tc.tile_pool(name="ptp", bufs=6))

        xin_cm = tc.tile_pool(name="xin", bufs=1)
        xin = xin_cm.__enter__()
        # x chunk split in column halves: wave-0 matmuls touching only
        # the first half start ~1us earlier per chunk (tile-granular
        # DMA semaphores would otherwise gate on the full 512KB)
        xfa = [xin.tile([128, S // 2], BF, tag=f"xfa{k}", name=f"xfa{k}")
               for k in range(KC)]
        xfb = [xin.tile([128, S // 2], BF, tag=f"xfb{k}", name=f"xfb{k}")
               for k in range(KC)]
        wkv = [xin.tile([128, 2 * NKV * HD], BF, tag=f"wkv{k}", name=f"wkv{k}")
               for k in range(KC)]

        def wk_col(k, lo, n):
            return wkv[k][:, lo:lo + n]

        def wv_col(k, lo, n):
            return wkv[k][:, 512 + lo:512 + lo + n]

        # whole-unit weight tiles for wave-0's Q0/Q2
        wq_w0 = {m: wqp.tile([128, KC * 128], BF, tag="wq", name=f"wqw{m}")
                 for m in (0, 2)}

        # ---- DMA issue order = priority. Two issues per x chunk; wq
        # whole-unit tiles first so wave 0's Q matmuls never stall the
        # in-order PE queue.
        for m in (0, 2):
            nc.sync.dma_start(out=wq_w0[m][:, :],
                              in_=wqTk[:, :, m * 128:(m + 1) * 128])
        for k in range(KC):
            nc.sync.dma_start(out=wkv[k][:, :],
                              in_=wkvT[k * 128:(k + 1) * 128, :])
            nc.sync.dma_start(out=xfa[k][:, :],
                              in_=xT[k * 128:(k + 1) * 128, 0:S // 2])
            nc.sync.dma_start(out=xfb[k][:, :],
                              in_=xT[k * 128:(k + 1) * 128, S // 2:S])

        def xcols(k, lo, n):
            # columns lo..lo+n of the permuted x^T chunk k
            assert lo + n <= S // 2 or lo >= S // 2
            return xfa[k][:, lo:lo + n] if lo < S // 2 \
                else xfb[k][:, lo - S // 2:lo - S // 2 + n]

        # row HD (partition 64) is the K=1 lhsT for the row-sum broadcast
        ones64 = T([HD + 1, 64], BF, name="ones64")
        nc.vector.memset(ones64[:, :], 1.0)
        ones128 = T([1, 128], BF, name="ones128")
        nc.vector.memset(ones128[:, :], 1.0)
        ones512 = T([1, TOK], BF, name="ones512")
        nc.vector.memset(ones512[:, :], 1.0)

        # kTg[nb*4+mt]: [128, 512] = K^T rows mt*128.. for key block nb
        # (kv heads 2mt at partitions 0-63, 2mt+1 at 64-127).
        # vg[c]: [128, 520] V_aug rows for key chunk c, ones at col
        # kh*65+64 of each kv head kh.
        kTg = [T([128, TOK], BF, name=f"kTg{i}") for i in range(16)]
        vg = [T([128, VW], BF, name=f"vg{c}") for c in range(NKC)]
        qTp = [T([128, TOK], BF, name=f"qTp{i}") for i in range(NH // 2)]
        attnT = [T([128, TOK], BF, name=f"attnT{t}") for t in range(NH // 2)]

        def q_fin(ps, m):
            st = stgB.tile([128, TOK], BF, tag="stg", name=f"stq{m}")
            nc.vector.tensor_copy(out=st[:, :], in_=ps[:, :])
            for j in range(2):
                h = 2 * m + j
                i, roff = q_slot(h)
                nc.sync.dma_start(out=qTp[i][roff:roff + 64, :],
                                  in_=st[j * 64:(j + 1) * 64, :])

        def v_fin(ps, c, h):
            # single strided copy (4 tiny copies would each pay the
            # ~0.2us DVE drain bubble and clog the in-order DVE queue)
            if h == 0:
                nc.vector.memset(vg[c][:, :], 1.0)
            dst = vg[c][:, 4 * h * (HD + 1):(4 * h + 4) * (HD + 1)]
            nc.vector.tensor_copy(
                out=dst.rearrange("p (g c) -> p g c", g=4)[:, :, 0:HD],
                in_=ps.rearrange("p (g c) -> p g c", g=4))

        # =============== wave 0: the minimal exp-gating work ============
        # 8 PSUM accumulation groups, contraction-chunk OUTER so the PE
        # streams 8 matmuls per arriving x chunk. Completes ~1.7us after
        # the last x chunk lands; first exp fires ~2us later.
        w0 = ([("k", 0, nb) for nb in range(TP)]
              + [("q", 0, None), ("q", 2, None)]
              + [("v", 0, 0), ("v", 1, 0)])
        with tc.tile_pool(name="accA", bufs=8, space="PSUM") as accA:
            pss = [accA.tile([128, TOK], F32, tag="acc", name=f"psA{i}")
                   for i in range(8)]
            for k in range(KC):
                for ps, (kind, a, b) in zip(pss, w0):
                    if kind == "k":
                        nc.tensor.matmul(
                            out=ps[:, :],
                            lhsT=wk_col(k, 0, 128),
                            rhs=xcols(k, b * TOK, TOK),
                            start=(k == 0), stop=(k == KC - 1))
                    elif kind == "q":
                        nc.tensor.matmul(
                            out=ps[:, :],
                            lhsT=wq_w0[a][:, k * 128:(k + 1) * 128],
                            rhs=xcols(k, 0, TOK),
                            start=(k == 0), stop=(k == KC - 1))
                    else:
                        nc.tensor.matmul(
                            out=ps[:, 0:256],
                            lhsT=xcols(k, a * 128, 128),
                            rhs=wv_col(k, 0, 256),
                            start=(k == 0), stop=(k == KC - 1))
            # evac in exp-gating order: kTg nb0, Q0, Q2, then the rest
            nc.vector.tensor_copy(out=kTg[0][:, :], in_=pss[0][:, :])
            q_fin(pss[4], 0)
            q_fin(pss[5], 2)
            for nb in range(1, TP):
                nc.vector.tensor_copy(out=kTg[nb * 4][:, :],
                                      in_=pss[nb][:, :])
            v_fin(pss[6][:, 0:256], 0, 0)
            v_fin(pss[7][:, 0:256], 1, 0)

        # =============== drip units (fed into the pair loop) ============
        # Each unit: (deadline_step, [(cost_ns, thunk), ...]).
        MM_NS = 216    # 512-col bf16 matmul streaming time
        VMM_NS = 112   # 256-col

        drip = deque()  # (deadline, cost_ns, thunk)
        _units = []     # (deadline, seq, [(cost, thunk), ...])

        def push_unit(deadline, cts):
            _units.append((deadline, len(_units), cts))

        def seal_units():
            # stable-sort by real deadline; pacing happens at pop time
            # via an adaptive quota (see the pair loop)
            total = 0
            for d, _, cts in sorted(_units, key=lambda u: (u[0], u[1])):
                for c, t in cts:
                    drip.append((d, c, t))
                    total += c
            return total

        # unit PSUM tiles MUST allocate lazily at pop time: the dps ring
        # orders its WAR handoffs by .tile() call order, which has to
        # match emission order (norm_bcast's pb tiles share the ring).
        def gen_q_chunk(m):
            """one whole-unit weight DMA, 16 matmuls, finisher."""
            box = {}
            ws = wqp.tile([128, KC * 128], BF, tag="wq", name=f"wqu{m}")
            def dm(m=m):
                nc.sync.dma_start(out=ws[:, :],
                                  in_=wqTk[:, :, m * 128:(m + 1) * 128])
            thunks = [(0, dm)]
            for k in range(KC):
                def mm(k=k, m=m):
                    if k == 0:
                        box["ps"] = dps.tile([128, TOK], F32, tag="dp",
                                             name=f"psq{m}")
                    nc.tensor.matmul(
                        out=box["ps"][:, :],
                        lhsT=ws[:, k * 128:(k + 1) * 128],
                        rhs=xcols(k, 0, TOK),
                        start=(k == 0), stop=(k == KC - 1))
                thunks.append((MM_NS, mm))
            thunks.append((0, lambda m=m: q_fin(box["ps"], m)))
            return thunks

        def gen_k_unit(mt, nb):
            box = {}
            thunks = []
            for k in range(KC):
                def mm(k=k, mt=mt, nb=nb):
                    if k == 0:
                        box["ps"] = dps.tile([128, TOK], F32, tag="dp",
                                             name=f"psk{nb}_{mt}")
                    nc.tensor.matmul(
                        out=box["ps"][:, :],
                        lhsT=wk_col(k, mt * 128, 128),
                        rhs=xcols(k, nb * TOK, TOK),
                        start=(k == 0), stop=(k == KC - 1))
                thunks.append((MM_NS, mm))
            def fin(mt=mt, nb=nb):
                nc.vector.tensor_copy(out=kTg[nb * 4 + mt][:, :],
                                      in_=box["ps"][:, :])
            thunks.append((0, fin))
            return thunks

        def gen_v_unit(c, h):
            box = {}
            thunks = []
            for k in range(KC):
                def mm(k=k, c=c, h=h):
                    if k == 0:
                        box["ps"] = dps.tile([128, TOK], F32, tag="dp",
                                             name=f"psv{c}_{h}")
                    nc.tensor.matmul(
                        out=box["ps"][:, 0:256],
                        lhsT=xcols(k, c * 128, 128),
                        rhs=wv_col(k, h * 256, 256),
                        start=(k == 0), stop=(k == KC - 1))
                thunks.append((VMM_NS, mm))
            thunks.append(
                (0, lambda c=c, h=h: v_fin(box["ps"][:, 0:256], c, h)))
            return thunks

        # Wo prefetch (DMA-only): queued when xin releases at step 118.
        # One [128, 2048] row-block DMA per kc (all 4 nt at once).
        wo_pre = {}
        wo2_box = []

        def gen_wo_prefetch():
            thunks = []
            def last():
                wl = wo2_box[0].tile([1, HID], BF, tag="wolast",
                                     bufs=1, name="wl")
                nc.sync.dma_start(out=wl[:, :], in_=woT[HID:HID + 1, :])
                for nt in range(4):
                    wo_pre[(nt, "last")] = wl[:, nt * 512:(nt + 1) * 512]
            thunks.append((0, last))
            for kc in range(KC):
                def f(kc=kc):
                    w = wo2_box[0].tile([128, HID], BF, tag="wo", bufs=16,
                                        name=f"wo{kc}")
                    nc.sync.dma_start(
                        out=w[:, :], in_=woT[kc * 128:(kc + 1) * 128, :])
                    for nt in range(4):
                        wo_pre[(nt, kc)] = w[:, nt * 512:(nt + 1) * 512]
                thunks.append((0, f))
            return thunks

        # =============== the pair loop ==================================
        pairs = []
        for g in range(0, NKV, 2):
            for j in range(GRP):
                pairs.append((g * GRP + j, (g + 1) * GRP + j))

        bc_box = [None]  # (pool, tag) for the row-sum broadcast PSUM
        ops_cm = tc.tile_pool(name="ops", bufs=2, space="PSUM")
        ops = ops_cm.__enter__()
        with tc.tile_pool(name="sps", bufs=2, space="PSUM") as sps, \
             tc.tile_pool(name="dps", bufs=2, space="PSUM") as dps:
            bc_box[0] = (dps, "dp")

            # drip queue in deadline order
            def push_q(d, m):
                ths = gen_q_chunk(m)
                # weight DMA leads its matmuls by ~4 steps so the 512KB
                # transfer never head-of-line blocks the PE queue
                push_unit(max(0, d - 4), [ths[0]])
                push_unit(d, ths[1:])

            for c in range(2, NKC):                      # V kv0-3 rest
                push_unit(c // 2 + 1, gen_v_unit(c, 0))
            push_q(13, 1)
            push_q(13, 3)
            for nb in range(TP):                         # K mt1
                push_unit(28 + 2 * nb, gen_k_unit(1, nb))
            push_q(29, 4)
            push_q(29, 6)
            push_q(44, 5)
            push_q(44, 7)
            for c in range(NKC):                         # V kv4-7
                push_unit(61 + c // 2, gen_v_unit(c, 1))
            for nb in range(TP):                         # K mt2
                push_unit(61 + 2 * nb, gen_k_unit(2, nb))
            push_q(61, 8)
            push_q(61, 10)
            push_q(76, 9)
            push_q(76, 11)
            for nb in range(TP):                         # K mt3
                push_unit(92 + 2 * nb, gen_k_unit(3, nb))
            push_q(93, 12)
            push_q(93, 14)
            push_q(105, 13)
            push_q(105, 15)
            rem_cost = seal_units()

            # Normalization for pair p staged across pair p+1's steps
            # (PSUM->SBUF copy, reciprocal, cast, PE ones-broadcast via
            # the dps ring, multiply) so the 3.3us DVE reciprocal never
            # blocks a PE-side consumer.
            aph_of = {}

            def norm_copy(p, ci):
                j = 0 if ci == 2 else 1
                h = pairs[p][j]
                po = po_of[p][j]
                aph = nrm.tile([HD + 1, TOK], F32, tag="aph", bufs=2,
                               name=f"aph{h}")
                nc.vector.tensor_copy(out=aph[:, :], in_=po[:, :])
                aph_of[(p, j)] = [aph, None, None]
                if ci == 4:
                    del po_of[p]

            def norm_recip(p, j):
                # DVE reciprocal is an iterative divide, and the row sum
                # lives on ONE partition: [1,512] costs 3.3us on a single
                # lane. Spread it to [128,4] via a reshaping SBUF->SBUF
                # DMA so all 128 lanes divide in parallel (~0.15us).
                h = pairs[p][j]
                ent = aph_of[(p, j)]
                z4 = nrm.tile([128, 4], F32, tag="z4", name=f"z4{h}")
                nc.sync.dma_start(out=z4[:, :], in_=ent[0][HD:HD + 1, :])
                r4 = nrm.tile([128, 4], F32, tag="r4", name=f"r4{h}")
                nc.vector.reciprocal(out=r4[:, :], in_=z4[:, :])
                rb4 = nrm.tile([128, 4], BF, tag="rb4", name=f"rb4{h}")
                nc.vector.tensor_copy(out=rb4[:, :], in_=r4[:, :])
                ent[1] = rb4

            def norm_cast(p, j):
                # gather the spread reciprocal back to [1,512] bf16
                h = pairs[p][j]
                ent = aph_of[(p, j)]
                rcpb = nrm.tile([HD + 1, TOK], BF, tag="rcpb", name=f"rb{h}")
                nc.sync.dma_start(out=rcpb[HD:HD + 1, :], in_=ent[1][:, :])
                ent[1] = rcpb

            def norm_bcast(p, j):
                h = pairs[p][j]
                ent = aph_of[(p, j)]
                pool, tag = bc_box[0]
                pb = pool.tile([128, TOK], F32, tag=tag, name=f"pb{h}")
                nc.tensor.matmul(out=pb[0:64, :], lhsT=ones64[HD:HD + 1, :],
                                 rhs=ent[1][HD:HD + 1, :],
                                 start=True, stop=True)
                rb = nrm.tile([64, TOK], BF, tag="rbb", name=f"rbb{h}")
                nc.vector.tensor_copy(out=rb[:, :], in_=pb[0:64, :])
                ent[2] = rb

            def norm_mul(p, j):
                h = pairs[p][j]
                ent = aph_of[(p, j)]
                t, half = h // 2, (h % 2) * 64
                if half == 0:
                    nc.vector.tensor_mul(out=attnT[t][0:64, :],
                                         in0=ent[0][0:HD, :],
                                         in1=ent[2][:, :])
                else:
                    ah = nrm.tile([64, TOK], BF, tag="ah", name=f"ah{h}")
                    nc.vector.tensor_mul(out=ah[:, :], in0=ent[0][0:HD, :],
                                         in1=ent[2][:, :])
                    nc.sync.dma_start(out=attnT[t][64:128, :], in_=ah[:, :])
                del aph_of[(p, j)]

            def norm_stage2(p, ci):
                if ci == 4:
                    norm_recip(p, 0)
                elif ci == 6:
                    norm_recip(p, 1)
                elif ci == 8:
                    norm_cast(p, 0)
                    norm_cast(p, 1)
                elif ci == 10:
                    norm_bcast(p, 0)
                elif ci == 12:
                    norm_bcast(p, 1)
                    norm_mul(p, 0)
                elif ci == 14:
                    norm_mul(p, 1)

            def emit_scores(pi, ci):
                hA, hB = pairs[pi]
                kt = (hA // GRP) // 2
                qiA, _ = q_slot(hA)
                qiB, _ = q_slot(hB)
                psA = sps.tile([128, 2 * TOK], F32, tag="ps",
                               name=f"psA{hA}_{ci}")
                psB = sps.tile([128, 2 * TOK], F32, tag="ps",
                               name=f"psB{hB}_{ci}")
                for dc in range(2):
                    c = ci + dc
                    nb, lc = c // 4, c % 4
                    kts = kTg[nb * 4 + kt]
                    nc.tensor.matmul(
                        out=psA[:, dc * TOK:(dc + 1) * TOK],
                        lhsT=kts[0:64, lc * 128:(lc + 1) * 128],
                        rhs=qTp[qiA][0:64, :], start=True, stop=True)
                ptA = ptp.tile([128, 2 * TOK], BF, tag="pt",
                               name=f"ptA{hA}_{ci}")
                nc.scalar.activation(
                    out=ptA[:, :], in_=psA[:, :],
                    func=mybir.ActivationFunctionType.Exp, scale=EXP_SCALE)
                for dc in range(2):
                    c = ci + dc
                    nb, lc = c // 4, c % 4
                    kts = kTg[nb * 4 + kt]
                    nc.tensor.matmul(
                        out=psB[:, dc * TOK:(dc + 1) * TOK],
                        lhsT=kts[64:128, lc * 128:(lc + 1) * 128],
                        rhs=qTp[qiB][64:128, :], start=True, stop=True)
                ptB = ptp.tile([128, 2 * TOK], BF, tag="pt",
                               name=f"ptB{hB}_{ci}")
                nc.scalar.activation(
                    out=ptB[:, :], in_=psB[:, :],
                    func=mybir.ActivationFunctionType.Exp, scale=EXP_SCALE)
                return ptA, ptB

            def emit_pv(pi, ci, ptA, ptB):
                hA, hB = pairs[pi]
                khA, khB = hA // GRP, hB // GRP
                poA, poB = po_of[pi]
                for dc in range(2):
                    c = ci + dc
                    nc.tensor.matmul(
                        out=poA[:, :],
                        lhsT=vg[c][:, khA * (HD + 1):(khA + 1) * (HD + 1)],
                        rhs=ptA[:, dc * TOK:(dc + 1) * TOK],
                        start=(c == 0), stop=(c == NKC - 1))
                    nc.tensor.matmul(
                        out=poB[:, :],
                        lhsT=vg[c][:, khB * (HD + 1):(khB + 1) * (HD + 1)],
                        rhs=ptB[:, dc * TOK:(dc + 1) * TOK],
                        start=(c == 0), stop=(c == NKC - 1))

            sched = [(pi, 2 * c2) for pi in range(len(pairs))
                     for c2 in range(NKC // 2)]
            po_of = {}
            inflight = deque()

            for s, (pi, ci) in enumerate(sched):
                hA, hB = pairs[pi]
                if ci == 0:
                    poA = ops.tile([HD + 1, TOK], F32, tag="po",
                                   name=f"poA{hA}")
                    poB = ops.tile([HD + 1, TOK], F32, tag="po",
                                   name=f"poB{hB}")
                    po_of[pi] = (poA, poB)
                # PV of step s-LAG first: it never waits, so it fills the
                # window where scores-A(s) stalls on exp(s-1) freeing the
                # score-PSUM ring (the PE queue is in-order).
                if len(inflight) >= LAG:
                    emit_pv(*inflight.popleft())
                ptA, ptB = emit_scores(pi, ci)
                inflight.append((pi, ci, ptA, ptB))
                if pi > 0:
                    if ci in (2, 4):
                        norm_copy(pi - 1, ci)
                    if ci >= 4:
                        norm_stage2(pi - 1, ci)
                # drip pops: real deadlines force correctness-critical
                # work; the adaptive quota spreads everything else so
                # the queue neither bursts (burying the next scores in
                # the in-order PE queue) nor runs dry (PE p-state drop).
                due = 0
                for dd, cc, _ in drip:
                    if dd > s + 30:
                        break
                    due += cc
                quota = max(rem_cost / max(1.0, 126.0 - s), due / 30.0)
                spent = 0
                while drip and (drip[0][0] <= s + 2 or spent < quota):
                    _, cost, th = drip.popleft()
                    th()
                    spent += cost
                    rem_cost -= cost
                # keep-warm fillers: an idle PE drops out of its max
                # p-state within ~1us and every matmul then runs ~1.6x
                # slow. On bare steps burn ~0.5us on K=1 junk matmuls.
                if spent < 400 and s > 4:
                    for f in range(2):
                        fp = dps.tile([128, TOK], F32, tag="dp",
                                      name=f"fil{s}_{f}")
                        nc.tensor.matmul(out=fp[:, :], lhsT=ones128[:, :],
                                         rhs=ones512[:, :],
                                         start=True, stop=True)
                # release xin at step 118 (all x/wk/wv consumers done by
                # ~110) and stream the Wo prefetch into the freed SBUF.
                if s == 118:
                    xin_cm.__exit__(None, None, None)
                    wo2_box.append(es.enter_context(
                        tc.tile_pool(name="wo2", bufs=1)))
                    for i, (cst, th) in enumerate(gen_wo_prefetch()):
                        drip.append((119 + i, cst, th))

            while inflight:
                emit_pv(*inflight.popleft())
            while drip:
                drip.popleft()[2]()

        # =============== phase E: output projection + bias ===========
        # sps/dps closed; ops stays open so pair 15's norm drain (which
        # reads po(15)) can overlap E's first 14 kc-groups — attnT[13]
        # and attnT[15] are the only pair-15-gated contraction chunks,
        # so they accumulate last.
        with tc.tile_pool(name="yps", bufs=6, space="PSUM") as yps, \
             tc.tile_pool(name="ystg", bufs=4) as ystg:
            bc_box[0] = (yps, "py")
            kc_order = list(range(13)) + [14, 13, 15]
            for nt in range(4):        # 4 output column blocks of 512
                wo_last = wo_pre[(nt, "last")]
                pys = [yps.tile([128, 512], F32, tag="py",
                                name=f"py{nt}_{i}") for i in range(4)]
                for idx, kc in enumerate(kc_order):
                    wo_t = wo_pre[(nt, kc)]
                    for mt in range(4):
                        nc.tensor.matmul(
                            out=pys[mt][:, :],
                            lhsT=attnT[kc][:, mt * 128:(mt + 1) * 128],
                            rhs=wo_t[:, :],
                            start=(idx == 0), stop=False)
                    if nt == 0 and idx == 13:
                        # pair-15 norm drain: DVE chain runs while the
                        # PE streams the kc-groups emitted above
                        for ci in (2, 4):
                            norm_copy(15, ci)
                        for ci in range(4, 16, 2):
                            norm_stage2(15, ci)
                for mt in range(4):    # bias via ones row, K=1 matmul
                    nc.tensor.matmul(
                        out=pys[mt][:, :], lhsT=ones128[:, :],
                        rhs=wo_last[:, :], start=False, stop=True)
                    ys = ystg.tile([128, 512], F32, tag="ys",
                                   name=f"ys{nt}_{mt}")
                    nc.vector.tensor_copy(out=ys[:, :], in_=pys[mt][:, :])
                    nc.sync.dma_start(
                        out=out[mt * 128:(mt + 1) * 128,
                                nt * 512:(nt + 1) * 512],
                        in_=ys[:, :])
        ops_cm.__exit__(None, None, None)

    nc.finalize()
    return nc


@functools.lru_cache(maxsize=1)
def _graph():
    return build_graph()


def make_in_maps(x, Wq, Wk, Wv, Wo, bo):
    bf16 = ml_dtypes.bfloat16
    x = np.asarray(x, np.float32)
    wqT = np.asarray(Wq, np.float32).T                    # [HID, HID]
    # pre-tiled so one [128, KC*128] DMA loads a whole Q chunk's weights
    wqTk = np.ascontiguousarray(
        wqT.reshape(KC, 128, HID).transpose(1, 0, 2)).astype(bf16)
    wkvT = np.ascontiguousarray(np.concatenate(
        [np.asarray(Wk, np.float32).T, np.asarray(Wv, np.float32).T],
        axis=1)).astype(bf16)                             # [HID, 1024]
    woT = np.concatenate(
        [np.asarray(Wo, np.float32).T,
         np.asarray(bo, np.float32)[None, :]], axis=0).astype(bf16)
    woT = np.ascontiguousarray(woT)
    in_maps = []
    for c in range(8):
        b, r = c // TP, c % TP
        # token permutation: own query block first, rest after (attention
        # is permutation-invariant over keys)
        perm = np.r_[r * TOK:(r + 1) * TOK, 0:r * TOK, (r + 1) * TOK:S]
        xT_c = np.ascontiguousarray(x[b].T[:, perm]).astype(bf16)
        in_maps.append(
            {"xT": xT_c, "wkvT": wkvT, "wqTk": wqTk, "woT": woT})
    return in_maps


def kernel(x, Wq, Wk, Wv, Wo, bo):
    nc = _graph()
    in_maps = make_in_maps(x, Wq, Wk, Wv, Wo, bo)
    res = run_bass_kernel_spmd(nc, in_maps, core_ids=list(range(8)))
    out = np.empty((B, S, HID), np.float32)
    for c in range(8):
        b, r = c // TP, c % TP
        out[b, r * TOK:(r + 1) * TOK, :] = np.asarray(
            res.results[c]["out"], np.float32)
    return out
